# revision 11
# baseline (speedup 1.0000x reference)
"""Causal multi-head attention block (GPT-style) for Trainium2, 8 NeuronCores.

Problem: x[4,2048,768] -> qkv = x@W_attn+b_attn -> 12-head causal attention
         -> y@W_proj+b_proj -> out[4,2048,768]   (fp32 I/O)

Sharding: 4 batches x 2 head-groups (6 heads each); core c = 2*b + g handles
batch b, heads 6g..6g+5. c_proj row-sharded; AllReduce(add) over core pairs.

v2 kernel — fp8 DoubleRow (DR) matmuls everywhere the error budget allows:
  1. Q,K projection: fp8e4 DR over d-chunk pairs (contraction 256/instr,
     0.5 cyc/row). W_attn[q,k] scaled by SA=16 on host so fp8 sees ~N(0,0.3).
     PSUM -> (DVE +bias, fp8 out) qraw[128,S] -> DMA relayout to [32,2,S]
     per head so scores can run DR with K=32x2=64 (exact head_dim).
  2. V projection: bf16 (V feeds y almost linearly -> needs > fp8 accuracy),
     then split v = v_hi + v_lo (both fp8e4); the AV matmul consumes
     [v_hi|v_lo] as the two DR k-tiles with the SAME pt tile (stride-0 dim)
     => exact-to-fp8-residual V at DR speed.
  3. Scores S^T[k,q] per 128-k block via one DR instr per head (K=32x2).
     Causal diagonal: 128-wide staircase chunks get a shared tril mask
     ([128,128], 0/-3e5) ADDED in PSUM by DVE before exp (no post-mask).
  4. exp on ACT (the bottleneck engine): wide [128,<=1024] calls, fp8 out.
  5. AV transposed: out y_u[q-part, 65] per (q-128-chunk, head, j):
     lhsT = pt (stationary, stride-0 doubled), rhs = [v_hi|v_lo]. The ones
     column of v_hi makes col 64 the softmax denominator n[q] -- a
     per-partition scalar: normalize = DVE recip[128,4] + one strided mul.
  6. y chunks [128 q, 128 hd] -> PE transpose (bf16) -> y^T for proj.
  7. proj: bf16, contraction over the core's 3 pair-tiles, PSUM->DRAM DMA
     (b_proj+b_v@W_proj folded on host; DVE add only if nonzero).
  8. AllReduce(add) in 2 chunks: rows [0:1024] mid-kernel (hidden),
     [1024:2048] as the tail.

The walrus build allows only one sync-wait per instruction; legalize_waits
hoists extras onto single-wait NOPs.
"""
import numpy as np
import ml_dtypes

import concourse.bass as bass
import concourse.tile as tile
from concourse import mybir
from concourse.bass_utils import run_bass_kernel_spmd
from concourse import mybir as mb

BF16 = mybir.dt.bfloat16
F8 = mybir.dt.float8e4
F32 = mybir.dt.float32
DR = mybir.MatmulPerfMode.DoubleRow

B, S, D = 4, 2048, 768
H, HD = 12, 64
G = 2                 # head groups
HL = H // G           # heads per core = 6
DL = HL * HD          # local head dims = 384
NP = HL // 2          # head pairs per core = 3
P = 128
QT = 512              # q tile
NI = S // QT          # 4 q tiles
NS = S // P           # 16 k/s tiles
NDT = D // P          # 6 d tiles
NDP = NDT // 2        # 3 d pair tiles
N_CORES = 8
SA = 16.0             # host scale on W_attn[q,k] before fp8
ESC = 0.125 / (SA * SA)   # exp scale undoing SA^2 and 1/sqrt(hd)
MASKC = -3.0e5
# diag remainder layout inside scR (bank-crossing-free): mi -> col offset
REM_OFF = {0: 0, 1: 512, 2: 384}
REM_W = {0: 384, 1: 256, 2: 128}


def _legalize_waits(nc):
    n_split = 0
    for f in nc.m.functions:
        for bb in f.blocks:
            insts = list(bb.instructions)
            out = []
            changed = False
            for inst in insts:
                si = inst.sync_info
                if si is not None:
                    waits = list(si.on_wait)
                    if len(waits) > 1:
                        for w in waits[:-1]:
                            nop = mb.InstNoOp(name=f"I-wsplit-{nc.next_id()}", ins=[], outs=[])
                            nop.engine = inst.engine
                            nop.sync_info = mb.SyncInfo(on_wait=[w], on_update=[])
                            out.append(nop)
                            n_split += 1
                        inst.sync_info = mb.SyncInfo(on_wait=[waits[-1]], on_update=list(si.on_update))
                        changed = True
                out.append(inst)
            if changed:
                bb.instructions = out
    return n_split


def _build(qk_bias: bool, proj_bias: bool):
    nc = bass.Bass("TRN2", target_bir_lowering=False, debug=False, num_devices=N_CORES)

    xq8 = nc.dram_tensor("xq8", [3 * P, 2 * S], F8, kind="ExternalInput").ap()
    xkb = nc.dram_tensor("xkb", [D, S], BF16, kind="ExternalInput").ap()
    wq8 = nc.dram_tensor("wq8", [3 * P, 2 * 2 * DL], F8, kind="ExternalInput").ap()
    wvb = nc.dram_tensor("wvb", [D, DL], BF16, kind="ExternalInput").ap()
    wpb = nc.dram_tensor("wpb", [DL, D], BF16, kind="ExternalInput").ap()
    ba8 = nc.dram_tensor("ba8", [P, 8], F32, kind="ExternalInput").ap()
    bpb = nc.dram_tensor("bpb", [P, D], F32, kind="ExternalInput").ap()
    mtri = nc.dram_tensor("mtri", [P, P], F32, kind="ExternalInput").ap()
    idn = nc.dram_tensor("idn", [P, P], BF16, kind="ExternalInput").ap()
    out = nc.dram_tensor("out", [S, D], F32, kind="ExternalOutput").ap()
    ar_in = nc.dram_tensor("ar_in", [S, D], F32).ap()
    ar_outs = [nc.dram_tensor(f"ar_out{c}", [S // 2, D], F32).ap() for c in range(2)]

    with tile.TileContext(nc) as tc:
        with (
            tc.tile_pool(name="wgt", bufs=1) as wpool,
            tc.tile_pool(name="qk", bufs=1) as qkpool,
            tc.tile_pool(name="vpr", bufs=1) as vpool,
            tc.tile_pool(name="pt", bufs=6) as ptpool,
            tc.tile_pool(name="yc", bufs=2) as ycpool,
            tc.tile_pool(name="ytp", bufs=2) as ytpool,
            tc.tile_pool(name="nrm", bufs=2) as nrmpool,
            tc.tile_pool(name="ob", bufs=3) as obpool,
            tc.tile_pool(name="scp", bufs=2, space="PSUM") as scpool,
            tc.tile_pool(name="avp", bufs=1, space="PSUM") as avpool,
            tc.tile_pool(name="trp", bufs=1, space="PSUM") as trpool,
            tc.tile_pool(name="prj", bufs=1, space="PSUM") as prpool,
        ):
            # ---- phase 0: load weights/constants ----
            xq = []   # fp8 x d-pair tiles [128, 2, S]
            for t in range(NDP):
                xt = wpool.tile([P, 2 * S], F8, tag=f"xq{t}")
                nc.sync.dma_start(xt[:, 0:S], xq8[bass.ts(t, P), 0:S])
                nc.sync.dma_start(xt[:, S:], xq8[bass.ts(t, P), S:])
                xq.append(xt)
            wq = []   # fp8 wa-qk d-pair tiles [128, 2, 768]
            for t in range(NDP):
                wt = wpool.tile([P, 2 * 2 * DL], F8, tag=f"wq{t}")
                nc.sync.dma_start(wt[:], wq8[bass.ts(t, P), :])
                wq.append(wt)
            xk = []   # bf16 x^T tiles [128, S] (V stationary)
            for t in range(NDT):
                xt = wpool.tile([P, S], BF16, tag=f"xk{t}")
                nc.sync.dma_start(xt[:, 0 : S // 2], xkb[bass.ts(t, P), 0 : S // 2])
                xk.append(xt)
            for t in range(NDT):
                nc.sync.dma_start(xk[t][:, S // 2 :], xkb[bass.ts(t, P), S // 2 :])
            wv = []
            for t in range(NDT):
                wt = wpool.tile([P, DL], BF16, tag=f"wv{t}")
                nc.sync.dma_start(wt[:], wvb[bass.ts(t, P), :])
                wv.append(wt)
            wpp = []
            for p in range(NP):
                wt = wpool.tile([P, D], BF16, tag=f"wp{p}")
                nc.sync.dma_start(wt[:], wpb[bass.ts(p, P), :])
                wpp.append(wt)
            ba_sb = wpool.tile([P, 8], F32, tag="ba")
            if qk_bias:
                nc.sync.dma_start(ba_sb[:], ba8[:])
            bp_sb = wpool.tile([P, D], F32, tag="bp")
            if proj_bias:
                nc.sync.dma_start(bp_sb[:], bpb[:])
            mt_sb = wpool.tile([P, P], F32, tag="mtri")
            nc.sync.dma_start(mt_sb[:], mtri[:])
            id_sb = wpool.tile([P, P], BF16, tag="idn")
            nc.sync.dma_start(id_sb[:], idn[:])
            # prewarm ScalarE's exp table
            warm_sb = wpool.tile([1, 2], F32, tag="warm")
            nc.vector.memset(warm_sb[:], 0.0)
            nc.scalar.activation(warm_sb[:, 1:2], warm_sb[:, 0:1],
                                 mybir.ActivationFunctionType.Exp)

            # ---- phase 1: Q,K -> fp8 [32,2,S]-layout tiles ----
            # qraw m-tiles 0..2 = Q pairs, 3..5 = K pairs, [128, S] fp8
            # (partitions: head A dims 0:64, head B dims 64:128)
            qt8 = [qkpool.tile([64, 2 * S], F8, tag=f"qt8{p}", name=f"qt8{p}") for p in range(NP)]
            kt8 = [qkpool.tile([64, 2 * S], F8, tag=f"kt8{p}", name=f"kt8{p}") for p in range(NP)]

            def emit_qk(m):
                qraw = qkpool.tile([P, S], F8, tag=f"qraw{m}")
                for n in range(S // QT):
                    ps = scpool.tile([P, 1024], F32, tag="sc")
                    for t in range(NDP):
                        nc.tensor.matmul(
                            ps[:, 0:QT],
                            lhsT=wq[t][:].rearrange("p (u m) -> p u m", u=2)[
                                :, :, bass.ts(m, P)],
                            rhs=xq[t][:].rearrange("p (u s) -> p u s", u=2)[
                                :, :, bass.ts(n, QT)],
                            start=(t == 0), stop=(t == NDP - 1), perf_mode=DR,
                        )
                    if qk_bias:
                        nc.vector.tensor_scalar_add(
                            qraw[:, bass.ts(n, QT)], ps[:, 0:QT], ba_sb[:, m : m + 1])
                    else:
                        nc.vector.tensor_copy(qraw[:, bass.ts(n, QT)], ps[:, 0:QT])
                # relayout [128, S] -> [32, 2, S] per head (A rows 0:32, B 32:64)
                dst = qt8[m] if m < NP else kt8[m - NP]
                for h in range(2):
                    for u in range(2):
                        nc.sync.dma_start(
                            dst[32 * h : 32 * h + 32, u * S : (u + 1) * S],
                            qraw[64 * h + 32 * u : 64 * h + 32 * u + 32, :],
                        )

            # V s-tiles: [128, 780] fp8 = [hi 6x65 | lo 6x65], col 64-of-65:
            # hi=1 (softmax denominator via ones trick), lo=0
            v_t = [None] * NS

            def emit_v(s):
                ps = prpool.tile([P, 512], F32, tag="prj")
                for t in range(NDT):
                    nc.tensor.matmul(
                        ps[:, 0:DL],
                        lhsT=xk[t][:, bass.ts(s, P)],
                        rhs=wv[t][:],
                        start=(t == 0), stop=(t == NDT - 1),
                    )
                vt = vpool.tile([P, HL * 65], BF16, tag=f"v{s}")
                vt3 = vt[:].rearrange("p (h x) -> p h x", h=HL)
                ps3 = ps[:, 0:DL].rearrange("p (h x) -> p h x", h=HL)
                nc.vector.tensor_copy(vt3[:, :, 0:HD], ps3)
                nc.vector.memset(vt3[:, :, HD : HD + 1], 1.0)
                v_t[s] = vt

            ytp_t = [None] * NP

            def q_ap(p, h, q0, n):
                # rhs AP [32, 2, n] at q offset q0 for head h of pair p
                return qt8[p][32 * h : 32 * h + 32, :].rearrange(
                    "p (u s) -> p u s", u=2)[:, :, q0 : q0 + n]

            def k_ap(p, h, j):
                return kt8[p][32 * h : 32 * h + 32, :].rearrange(
                    "p (u s) -> p u s", u=2)[:, :, bass.ts(j, P)]

            def emit_attn(i, p):
                av = avpool.tile([P, 1024], F32, tag="av")
                av_started = [False, False]

                def av_mm(h, cc, j, pt_ap, stop):
                    # start=True zero-fills the whole PSUM bank, so exactly
                    # the first matmul touching each head's bank carries it
                    hh = 2 * p + h
                    nc.tensor.matmul(
                        av[:, 512 * h + cc * 65 : 512 * h + cc * 65 + 65],
                        lhsT=pt_ap,
                        rhs=v_t[j][:, 65 * hh : 65 * hh + 65],
                        start=not av_started[h], stop=stop,
                        skip_group_check=True,
                    )
                    av_started[h] = True

                # off-diagonal full groups (pairs of j blocks)
                for g in range(2 * i):
                    j0, j1 = 2 * g, 2 * g + 1
                    pts = []
                    for h in range(2):
                        sc = scpool.tile([P, 1024], F32, tag="sc")
                        for jj, off in ((j0, 0), (j1, QT)):
                            nc.tensor.matmul(
                                sc[:, off : off + QT],
                                lhsT=k_ap(p, h, jj),
                                rhs=q_ap(p, h, i * QT, QT),
                                start=True, stop=True, perf_mode=DR,
                            )
                        pt = ptpool.tile([P, 1024], BF16, tag="pt")
                        nc.scalar.activation(pt[:], sc[:],
                                             mybir.ActivationFunctionType.Exp,
                                             scale=ESC)
                        pts.append(pt)
                    for h in range(2):
                        for cc in range(4):
                            for jj, off in ((j0, 0), (j1, QT)):
                                av_mm(h, cc, jj,
                                      pts[h][:, off + cc * P : off + cc * P + P],
                                      stop=False)

                # diagonal: 8 staircase chunks (A 0:512 | B 512:1024) + mask
                scd = scpool.tile([P, 1024], F32, tag="sc")
                for h in range(2):
                    for mi in range(4):
                        nc.tensor.matmul(
                            scd[:, h * QT + mi * P : h * QT + mi * P + P],
                            lhsT=k_ap(p, h, 4 * i + mi),
                            rhs=q_ap(p, h, i * QT + mi * P, P),
                            start=(mi == 0), stop=True, perf_mode=DR,
                            skip_group_check=True,
                        )
                scd8 = scd[:].rearrange("p (c n) -> p c n", c=8)
                nc.vector.tensor_add(
                    scd8, scd8, mt_sb[:].unsqueeze(1).broadcast_to((P, 8, P)))
                ptD = ptpool.tile([P, 1024], BF16, tag="pt")
                nc.scalar.activation(ptD[:], scd[:],
                                     mybir.ActivationFunctionType.Exp, scale=ESC)
                # diagonal remainders (mask-free): mi0@0 w384, mi2@384 w128,
                # mi1@512 w256  (bank-crossing-free packing)
                ptR = []
                for h in range(2):
                    scr = scpool.tile([P, 1024], F32, tag="sc")
                    for mi in range(3):
                        w = REM_W[mi]
                        off = REM_OFF[mi]
                        nc.tensor.matmul(
                            scr[:, off : off + w],
                            lhsT=k_ap(p, h, 4 * i + mi),
                            rhs=q_ap(p, h, i * QT + mi * P + P, w),
                            start=(mi < 2), stop=True, perf_mode=DR,
                            skip_group_check=True,
                        )
                    ptr = ptpool.tile([P, 1024], BF16, tag="pt")
                    nc.scalar.activation(ptr[:, 0:768], scr[:, 0:768],
                                         mybir.ActivationFunctionType.Exp,
                                         scale=ESC)
                    ptR.append(ptr)
                for h in range(2):
                    for cc in range(4):
                        for mi in range(cc):
                            av_mm(h, cc, 4 * i + mi,
                                  ptR[h][:, REM_OFF[mi] + (cc - mi - 1) * P :
                                         REM_OFF[mi] + (cc - mi) * P],
                                  stop=False)
                        av_mm(h, cc, 4 * i + cc,
                              ptD[:, h * QT + cc * P : h * QT + cc * P + P],
                              stop=True)

                # normalize (per-partition scalar) + build y chunks
                rc = nrmpool.tile([P, 8], F32, tag="rc")
                yc = ycpool.tile([P, QT], BF16, tag="yc")
                yc4 = yc[:].rearrange("p (c n) -> p c n", c=4)
                for h in range(2):
                    av4 = av[:, 512 * h : 512 * h + 260].rearrange(
                        "p (c n) -> p c n", c=4)
                    nc.vector.reciprocal(
                        rc[:, 4 * h : 4 * h + 4].unsqueeze(2), av4[:, :, 64:65])
                    nc.vector.tensor_mul(
                        yc4[:, :, 64 * h : 64 * h + 64],
                        av4[:, :, 0:64],
                        rc[:, 4 * h : 4 * h + 4].unsqueeze(2).broadcast_to((P, 4, 64)),
                    )
                ytp = ytpool.tile([P, QT], BF16, tag=f"ytp{p}")
                tp = trpool.tile([P, 1024], BF16, tag="tr")
                for cc in range(4):
                    nc.tensor.matmul(
                        tp[:, cc * P : cc * P + P], yc[:, cc * P : cc * P + P],
                        id_sb[:], is_transpose=True,
                        start=(cc == 0), stop=True, skip_group_check=True)
                nc.vector.tensor_copy(ytp[:], tp[:, 0:QT])
                ytp_t[p] = ytp

            def emit_copy(c, r):
                rr = r - c * (S // 2)
                oc = obpool.tile([P, D], F32, tag="oc")
                nc.sync.dma_start(oc[:], ar_outs[c][rr : rr + P, :])
                nc.sync.dma_start(out[r : r + P, :], oc[:])

            def emit_ar(c):
                r0 = c * (S // 2)
                nc.gpsimd.collective_compute(
                    "AllReduce",
                    mybir.AluOpType.add,
                    replica_groups=[[0, 1], [2, 3], [4, 5], [6, 7]],
                    ins=[ar_in[r0 : r0 + S // 2, :].opt()],
                    outs=[ar_outs[c][:].opt()],
                )

            def emit_proj(i):
                for ss in range(4):
                    row = i * QT + ss * P
                    if proj_bias:
                        ps = prpool.tile([P, D], F32, tag="prjw")
                        for half in range(2):
                            for p in range(NP):
                                nc.tensor.matmul(
                                    ps[:, half * DL : half * DL + DL],
                                    lhsT=ytp_t[p][:, ss * P : ss * P + P],
                                    rhs=wpp[p][:, half * DL : half * DL + DL],
                                    start=(p == 0), stop=(p == NP - 1),
                                )
                        ob = obpool.tile([P, D], F32, tag="ob")
                        nc.vector.tensor_add(ob[:], ps[:], bp_sb[:])
                        nc.sync.dma_start(ar_in[row : row + P, :], ob[:])
                    else:
                        ob = obpool.tile([P, D], F32, tag="ob")
                        for half in range(2):
                            ps = prpool.tile([P, 512], F32, tag="prj")
                            for p in range(NP):
                                nc.tensor.matmul(
                                    ps[:, 0:DL],
                                    lhsT=ytp_t[p][:, ss * P : ss * P + P],
                                    rhs=wpp[p][:, half * DL : half * DL + DL],
                                    start=(p == 0), stop=(p == NP - 1),
                                )
                            nc.vector.tensor_copy(
                                ob[:, half * DL : half * DL + DL], ps[:, 0:DL])
                        nc.sync.dma_start(ar_in[row : row + P, :], ob[:])

            # ---- main schedule ----
            for m in range(2 * NP):
                emit_qk(m)
            for s in range(4):
                emit_v(s)
            for i in range(NI):
                for p in range(NP):
                    emit_attn(i, p)
                if i + 1 < NI:
                    for s in range(4 * (i + 1), 4 * (i + 2)):
                        emit_v(s)
                emit_proj(i)
                if i == 1:
                    emit_ar(0)
                if i == 3:
                    for r in range(0, S // 2, P):
                        emit_copy(0, r)
                    emit_ar(1)
                    for r in range(S // 2, S, P):
                        emit_copy(1, r)

    _legalize_waits(nc)
    return nc


_NC_CACHE = {}


def _get_nc(qk_bias=False, proj_bias=False):
    key = (qk_bias, proj_bias)
    if key not in _NC_CACHE:
        _NC_CACHE[key] = _build(qk_bias, proj_bias)
    return _NC_CACHE[key]


def _prep_inputs(x, W_attn, b_attn, W_proj, b_proj):
    bf = ml_dtypes.bfloat16
    f8 = ml_dtypes.float8_e4m3
    x = np.asarray(x, np.float32)
    W_attn = np.asarray(W_attn, np.float32)
    b_attn = np.asarray(b_attn, np.float32)
    W_proj = np.asarray(W_proj, np.float32)
    b_proj = np.asarray(b_proj, np.float32)

    k_idx = np.arange(P)[:, None]
    q_idx = np.arange(P)[None, :]
    mtri = np.where(q_idx >= k_idx, 0.0, MASKC).astype(np.float32)
    idn = np.eye(P).astype(bf)

    Wq = W_attn[:, 0:D]
    Wk = W_attn[:, D : 2 * D]
    Wv = W_attn[:, 2 * D :]

    in_maps = []
    meta = {}
    for c in range(N_CORES):
        b, g = divmod(c, 2)
        cols = slice(DL * g, DL * g + DL)
        xb = x[b]                                   # [S, D]
        # fp8 x in d-pair layout [3*128, 2*S]
        xq8 = np.empty((3 * P, 2 * S), f8)
        for t in range(NDP):
            for u in range(2):
                xq8[t * P : (t + 1) * P, u * S : (u + 1) * S] = (
                    xb[:, 256 * t + P * u : 256 * t + P * u + P].T.astype(f8))
        xkb = np.ascontiguousarray(xb.T).astype(bf)
        # fp8 W_attn q,k (scaled) in d-pair layout [3*128, 2*768]
        wa_qk = np.concatenate([Wq[:, cols], Wk[:, cols]], axis=1) * SA  # [D, 768]
        wq8 = np.empty((3 * P, 2 * 2 * DL), f8)
        for t in range(NDP):
            for u in range(2):
                wq8[t * P : (t + 1) * P, u * 2 * DL : (u + 1) * 2 * DL] = (
                    wa_qk[256 * t + P * u : 256 * t + P * u + P, :].astype(f8))
        wvb = np.ascontiguousarray(Wv[:, cols]).astype(bf)
        wpb = np.ascontiguousarray(W_proj[cols, :]).astype(bf)
        # qk bias (scaled): ba8[p, m] = SA * b[col m*128+p]
        ba_qk = np.concatenate([b_attn[0:D][cols], b_attn[D : 2 * D][cols]]) * SA
        ba8 = np.zeros((P, 8), np.float32)
        ba8[:, :6] = ba_qk.reshape(6, P).T
        # b_proj (+ v-bias folded) added once per row: only group 0 carries it
        bv = b_attn[2 * D :][cols]
        bp_eff = bv @ W_proj[cols, :] + (b_proj if g == 0 else 0.0)
        bpb = np.ascontiguousarray(
            np.broadcast_to(bp_eff.astype(np.float32), (P, D)))
        in_maps.append({
            "xq8": xq8, "xkb": xkb, "wq8": wq8, "wvb": wvb, "wpb": wpb,
            "ba8": ba8, "bpb": bpb, "mtri": mtri, "idn": idn,
        })
        meta.setdefault("qk_bias", bool(np.any(ba_qk != 0.0)))
        meta["proj_bias"] = meta.get("proj_bias", False) or bool(
            np.any(bp_eff != 0.0))
    return in_maps, meta


def kernel(x, W_attn, b_attn, W_proj, b_proj):
    in_maps, meta = _prep_inputs(x, W_attn, b_attn, W_proj, b_proj)
    nc = _get_nc(meta["qk_bias"], meta["proj_bias"])
    res = run_bass_kernel_spmd(nc, in_maps, list(range(N_CORES)))
    out = np.stack([res.results[2 * b]["out"] for b in range(B)])
    return out.astype(np.float32)


# revision 14
# speedup vs baseline: 1.1376x; 1.1376x over previous
"""Causal multi-head attention block (GPT-style) for Trainium2, 8 NeuronCores.

Problem: x[4,2048,768] -> qkv = x@W_attn+b_attn -> 12-head causal attention
         -> y@W_proj+b_proj -> out[4,2048,768]   (fp32 I/O)

Sharding: 4 batches x 2 head-groups (6 heads each); core c = 2*b + g handles
batch b, heads 6g..6g+5. c_proj row-sharded; AllReduce(add) over core pairs.

v2 kernel — fp8 DoubleRow (DR) matmuls everywhere the error budget allows:
  1. Q,K projection: fp8e4 DR over d-chunk pairs (contraction 256/instr,
     0.5 cyc/row). W_attn[q,k] scaled by SA=16 on host so fp8 sees ~N(0,0.3).
     PSUM -> (DVE +bias, fp8 out) qraw[128,S] -> DMA relayout to [32,2,S]
     per head so scores can run DR with K=32x2=64 (exact head_dim).
  2. V projection: bf16 (V feeds y almost linearly -> needs > fp8 accuracy),
     then split v = v_hi + v_lo (both fp8e4); the AV matmul consumes
     [v_hi|v_lo] as the two DR k-tiles with the SAME pt tile (stride-0 dim)
     => exact-to-fp8-residual V at DR speed.
  3. Scores S^T[k,q] per 128-k block via one DR instr per head (K=32x2).
     Causal diagonal: 128-wide staircase chunks get a shared tril mask
     ([128,128], 0/-3e5) ADDED in PSUM by DVE before exp (no post-mask).
  4. exp on ACT (the bottleneck engine): wide [128,<=1024] calls, fp8 out.
  5. AV transposed: out y_u[q-part, 65] per (q-128-chunk, head, j):
     lhsT = pt (stationary, stride-0 doubled), rhs = [v_hi|v_lo]. The ones
     column of v_hi makes col 64 the softmax denominator n[q] -- a
     per-partition scalar: normalize = DVE recip[128,4] + one strided mul.
  6. y chunks [128 q, 128 hd] -> PE transpose (bf16) -> y^T for proj.
  7. proj: bf16, contraction over the core's 3 pair-tiles, PSUM->DRAM DMA
     (b_proj+b_v@W_proj folded on host; DVE add only if nonzero).
  8. AllReduce(add) in 2 chunks: rows [0:1024] mid-kernel (hidden),
     [1024:2048] as the tail.

The walrus build allows only one sync-wait per instruction; legalize_waits
hoists extras onto single-wait NOPs.
"""
import numpy as np
import ml_dtypes

import concourse.bass as bass
import concourse.tile as tile
from concourse import mybir
from concourse.bass_utils import run_bass_kernel_spmd
from concourse import mybir as mb

BF16 = mybir.dt.bfloat16
F8 = mybir.dt.float8e4
F32 = mybir.dt.float32
DR = mybir.MatmulPerfMode.DoubleRow

B, S, D = 4, 2048, 768
H, HD = 12, 64
G = 2                 # head groups
HL = H // G           # heads per core = 6
DL = HL * HD          # local head dims = 384
NP = HL // 2          # head pairs per core = 3
P = 128
QT = 512              # q tile
NI = S // QT          # 4 q tiles
NS = S // P           # 16 k/s tiles
NDT = D // P          # 6 d tiles
NDP = NDT // 2        # 3 d pair tiles
N_CORES = 8
SA = 16.0             # host scale on W_attn[q,k] before fp8
ESC = 0.125 / (SA * SA)   # exp scale undoing SA^2 and 1/sqrt(hd)
MASKC = -3.0e5
# diag remainder layout inside scR (bank-crossing-free): mi -> col offset
REM_OFF = {0: 0, 1: 512, 2: 384}
REM_W = {0: 384, 1: 256, 2: 128}


def _legalize_waits(nc):
    n_split = 0
    for f in nc.m.functions:
        for bb in f.blocks:
            insts = list(bb.instructions)
            out = []
            changed = False
            for inst in insts:
                si = inst.sync_info
                if si is not None:
                    waits = list(si.on_wait)
                    if len(waits) > 1:
                        for w in waits[:-1]:
                            nop = mb.InstNoOp(name=f"I-wsplit-{nc.next_id()}", ins=[], outs=[])
                            nop.engine = inst.engine
                            nop.sync_info = mb.SyncInfo(on_wait=[w], on_update=[])
                            out.append(nop)
                            n_split += 1
                        inst.sync_info = mb.SyncInfo(on_wait=[waits[-1]], on_update=list(si.on_update))
                        changed = True
                out.append(inst)
            if changed:
                bb.instructions = out
    return n_split


def _build(qk_bias: bool, proj_bias: bool):
    nc = bass.Bass("TRN2", target_bir_lowering=False, debug=False, num_devices=N_CORES)

    xq8 = nc.dram_tensor("xq8", [3 * P, 2 * S], F8, kind="ExternalInput").ap()
    xkb = nc.dram_tensor("xkb", [D, S], BF16, kind="ExternalInput").ap()
    wq8 = nc.dram_tensor("wq8", [3 * P, 2 * 2 * DL], F8, kind="ExternalInput").ap()
    wvb = nc.dram_tensor("wvb", [D, DL], BF16, kind="ExternalInput").ap()
    wpb = nc.dram_tensor("wpb", [DL, D], BF16, kind="ExternalInput").ap()
    ba8 = nc.dram_tensor("ba8", [P, 8], F32, kind="ExternalInput").ap()
    bpb = nc.dram_tensor("bpb", [P, D], F32, kind="ExternalInput").ap()
    mtri = nc.dram_tensor("mtri", [P, P], BF16, kind="ExternalInput").ap()
    idn = nc.dram_tensor("idn", [P, P], BF16, kind="ExternalInput").ap()
    outpart = nc.dram_tensor("outpart", [S // 2, D], F32, kind="ExternalOutput").ap()
    ar_in = nc.dram_tensor("ar_in", [S, D], F32).ap()
    rs_buf = nc.dram_tensor("rs_buf", [S // 2, D], F32).ap()

    with tile.TileContext(nc) as tc:
        with (
            tc.tile_pool(name="wgt", bufs=1) as wpool,
            tc.tile_pool(name="qk", bufs=1) as qkpool,
            tc.tile_pool(name="vpr", bufs=1) as vpool,
            tc.tile_pool(name="pt", bufs=6) as ptpool,
            tc.tile_pool(name="yc", bufs=2) as ycpool,
            tc.tile_pool(name="ytp", bufs=2) as ytpool,
            tc.tile_pool(name="nrm", bufs=2) as nrmpool,
            tc.tile_pool(name="ob", bufs=3) as obpool,
            tc.tile_pool(name="scp", bufs=2, space="PSUM") as scpool,
            tc.tile_pool(name="avp", bufs=1, space="PSUM") as avpool,
            tc.tile_pool(name="trp", bufs=1, space="PSUM") as trpool,
            tc.tile_pool(name="prj", bufs=1, space="PSUM") as prpool,
        ):
            # ---- phase 0: load weights/constants ----
            xq = []   # fp8 x d-pair tiles [128, 2, S]
            for t in range(NDP):
                xt = wpool.tile([P, 2 * S], F8, tag=f"xq{t}")
                nc.sync.dma_start(xt[:, 0:S], xq8[bass.ts(t, P), 0:S])
                nc.sync.dma_start(xt[:, S:], xq8[bass.ts(t, P), S:])
                xq.append(xt)
            wq = []   # fp8 wa-qk d-pair tiles [128, 2, 768]
            for t in range(NDP):
                wt = wpool.tile([P, 2 * 2 * DL], F8, tag=f"wq{t}")
                nc.sync.dma_start(wt[:], wq8[bass.ts(t, P), :])
                wq.append(wt)
            xk = []   # bf16 x^T tiles [128, S] (V stationary)
            for t in range(NDT):
                xt = wpool.tile([P, S], BF16, tag=f"xk{t}")
                nc.sync.dma_start(xt[:, 0 : S // 2], xkb[bass.ts(t, P), 0 : S // 2])
                xk.append(xt)
            for t in range(NDT):
                nc.sync.dma_start(xk[t][:, S // 2 :], xkb[bass.ts(t, P), S // 2 :])
            wv = []
            for t in range(NDT):
                wt = wpool.tile([P, DL], BF16, tag=f"wv{t}")
                nc.sync.dma_start(wt[:], wvb[bass.ts(t, P), :])
                wv.append(wt)
            wpp = []
            for p in range(NP):
                wt = wpool.tile([P, D], BF16, tag=f"wp{p}")
                nc.sync.dma_start(wt[:], wpb[bass.ts(p, P), :])
                wpp.append(wt)
            ba_sb = wpool.tile([P, 8], F32, tag="ba")
            if qk_bias:
                nc.sync.dma_start(ba_sb[:], ba8[:])
            bp_sb = wpool.tile([P, D], F32, tag="bp")
            if proj_bias:
                nc.sync.dma_start(bp_sb[:], bpb[:])
            mt_sb = wpool.tile([P, P], BF16, tag="mtri")
            nc.sync.dma_start(mt_sb[:], mtri[:])
            id_sb = wpool.tile([P, P], BF16, tag="idn")
            nc.sync.dma_start(id_sb[:], idn[:])
            # prewarm ScalarE's exp table
            warm_sb = wpool.tile([1, 2], F32, tag="warm")
            nc.vector.memset(warm_sb[:], 0.0)
            nc.scalar.activation(warm_sb[:, 1:2], warm_sb[:, 0:1],
                                 mybir.ActivationFunctionType.Exp)

            # ---- phase 1: Q,K -> fp8 [32,2,S]-layout tiles ----
            # qraw m-tiles 0..2 = Q pairs, 3..5 = K pairs, [128, S] fp8
            # (partitions: head A dims 0:64, head B dims 64:128)
            qt8 = [qkpool.tile([64, 2 * S], F8, tag=f"qt8{p}", name=f"qt8{p}") for p in range(NP)]
            kt8 = [qkpool.tile([64, 2 * S], F8, tag=f"kt8{p}", name=f"kt8{p}") for p in range(NP)]

            def emit_qk(m):
                qraw = qkpool.tile([P, S], F8, tag=f"qraw{m}")
                for n in range(S // QT):
                    ps = scpool.tile([P, 1024], F32, tag="sc")
                    for t in range(NDP):
                        nc.tensor.matmul(
                            ps[:, 0:QT],
                            lhsT=wq[t][:].rearrange("p (u m) -> p u m", u=2)[
                                :, :, bass.ts(m, P)],
                            rhs=xq[t][:].rearrange("p (u s) -> p u s", u=2)[
                                :, :, bass.ts(n, QT)],
                            start=(t == 0), stop=(t == NDP - 1), perf_mode=DR,
                        )
                    if qk_bias:
                        nc.vector.tensor_scalar_add(
                            qraw[:, bass.ts(n, QT)], ps[:, 0:QT], ba_sb[:, m : m + 1])
                    else:
                        nc.vector.tensor_copy(qraw[:, bass.ts(n, QT)], ps[:, 0:QT])
                # relayout [128, S] -> [32, 2, S] per head (A rows 0:32, B 32:64)
                dst = qt8[m] if m < NP else kt8[m - NP]
                for h in range(2):
                    for u in range(2):
                        nc.sync.dma_start(
                            dst[32 * h : 32 * h + 32, u * S : (u + 1) * S],
                            qraw[64 * h + 32 * u : 64 * h + 32 * u + 32, :],
                        )

            # V s-tiles: [128, 780] fp8 = [hi 6x65 | lo 6x65], col 64-of-65:
            # hi=1 (softmax denominator via ones trick), lo=0
            v_t = [None] * NS

            def emit_v(s):
                ps = prpool.tile([P, 512], F32, tag="prj")
                for t in range(NDT):
                    nc.tensor.matmul(
                        ps[:, 0:DL],
                        lhsT=xk[t][:, bass.ts(s, P)],
                        rhs=wv[t][:],
                        start=(t == 0), stop=(t == NDT - 1),
                    )
                vt = vpool.tile([P, HL * 65], BF16, tag=f"v{s}")
                vt3 = vt[:].rearrange("p (h x) -> p h x", h=HL)
                ps3 = ps[:, 0:DL].rearrange("p (h x) -> p h x", h=HL)
                nc.vector.tensor_copy(vt3[:, :, 0:HD], ps3)
                nc.vector.memset(vt3[:, :, HD : HD + 1], 1.0)
                v_t[s] = vt

            ytp_t = [None] * NP

            def q_ap(p, h, q0, n):
                # rhs AP [32, 2, n] at q offset q0 for head h of pair p
                return qt8[p][32 * h : 32 * h + 32, :].rearrange(
                    "p (u s) -> p u s", u=2)[:, :, q0 : q0 + n]

            def k_ap(p, h, j):
                return kt8[p][32 * h : 32 * h + 32, :].rearrange(
                    "p (u s) -> p u s", u=2)[:, :, bass.ts(j, P)]

            def emit_attn(i, p):
                av = avpool.tile([P, 1024], F32, tag="av")
                av_started = [False, False]

                def av_mm(h, cc, j, pt_ap, stop):
                    # start=True zero-fills the whole PSUM bank, so exactly
                    # the first matmul touching each head's bank carries it
                    hh = 2 * p + h
                    nc.tensor.matmul(
                        av[:, 512 * h + cc * 65 : 512 * h + cc * 65 + 65],
                        lhsT=pt_ap,
                        rhs=v_t[j][:, 65 * hh : 65 * hh + 65],
                        start=not av_started[h], stop=stop,
                        skip_group_check=True,
                    )
                    av_started[h] = True

                # off-diagonal full groups (pairs of j blocks)
                for g in range(2 * i):
                    j0, j1 = 2 * g, 2 * g + 1
                    pts = []
                    for h in range(2):
                        sc = scpool.tile([P, 1024], F32, tag="sc")
                        for jj, off in ((j0, 0), (j1, QT)):
                            nc.tensor.matmul(
                                sc[:, off : off + QT],
                                lhsT=k_ap(p, h, jj),
                                rhs=q_ap(p, h, i * QT, QT),
                                start=True, stop=True, perf_mode=DR,
                            )
                        pt = ptpool.tile([P, 1024], BF16, tag="pt")
                        nc.scalar.activation(pt[:], sc[:],
                                             mybir.ActivationFunctionType.Exp,
                                             scale=ESC)
                        pts.append(pt)
                    for h in range(2):
                        for cc in range(4):
                            for jj, off in ((j0, 0), (j1, QT)):
                                av_mm(h, cc, jj,
                                      pts[h][:, off + cc * P : off + cc * P + P],
                                      stop=False)

                # diagonal: 8 staircase chunks (A 0:512 | B 512:1024) + mask
                scd = scpool.tile([P, 1024], F32, tag="sc")
                for h in range(2):
                    for mi in range(4):
                        nc.tensor.matmul(
                            scd[:, h * QT + mi * P : h * QT + mi * P + P],
                            lhsT=k_ap(p, h, 4 * i + mi),
                            rhs=q_ap(p, h, i * QT + mi * P, P),
                            start=(mi == 0), stop=False, perf_mode=DR,
                            skip_group_check=True,
                        )
                for h in range(2):
                    for mi in range(4):
                        # += mtri via PE: idn^T @ mtri = mtri (keeps the mask
                        # off DVE so exp never waits on the vector queue)
                        nc.tensor.matmul(
                            scd[:, h * QT + mi * P : h * QT + mi * P + P],
                            lhsT=id_sb[:], rhs=mt_sb[:],
                            start=False, stop=True, skip_group_check=True,
                        )
                ptD = ptpool.tile([P, 1024], BF16, tag="pt")
                nc.scalar.activation(ptD[:], scd[:],
                                     mybir.ActivationFunctionType.Exp, scale=ESC)
                # diagonal remainders (mask-free): mi0@0 w384, mi2@384 w128,
                # mi1@512 w256  (bank-crossing-free packing)
                ptR = []
                for h in range(2):
                    scr = scpool.tile([P, 1024], F32, tag="sc")
                    for mi in range(3):
                        w = REM_W[mi]
                        off = REM_OFF[mi]
                        nc.tensor.matmul(
                            scr[:, off : off + w],
                            lhsT=k_ap(p, h, 4 * i + mi),
                            rhs=q_ap(p, h, i * QT + mi * P + P, w),
                            start=(mi < 2), stop=True, perf_mode=DR,
                            skip_group_check=True,
                        )
                    ptr = ptpool.tile([P, 1024], BF16, tag="pt")
                    nc.scalar.activation(ptr[:, 0:768], scr[:, 0:768],
                                         mybir.ActivationFunctionType.Exp,
                                         scale=ESC)
                    ptR.append(ptr)
                for h in range(2):
                    for cc in range(4):
                        for mi in range(cc):
                            av_mm(h, cc, 4 * i + mi,
                                  ptR[h][:, REM_OFF[mi] + (cc - mi - 1) * P :
                                         REM_OFF[mi] + (cc - mi) * P],
                                  stop=False)
                        av_mm(h, cc, 4 * i + cc,
                              ptD[:, h * QT + cc * P : h * QT + cc * P + P],
                              stop=True)

                # normalize (per-partition scalar) + build y chunks
                rc = nrmpool.tile([P, 8], F32, tag="rc")
                yc = ycpool.tile([P, QT], BF16, tag="yc")
                yc4 = yc[:].rearrange("p (c n) -> p c n", c=4)
                for h in range(2):
                    av4 = av[:, 512 * h : 512 * h + 260].rearrange(
                        "p (c n) -> p c n", c=4)
                    nc.vector.reciprocal(
                        rc[:, 4 * h : 4 * h + 4].unsqueeze(2), av4[:, :, 64:65])
                    nc.vector.tensor_mul(
                        yc4[:, :, 64 * h : 64 * h + 64],
                        av4[:, :, 0:64],
                        rc[:, 4 * h : 4 * h + 4].unsqueeze(2).broadcast_to((P, 4, 64)),
                    )
                ytp = ytpool.tile([P, QT], BF16, tag=f"ytp{p}")
                tp = trpool.tile([P, 1024], BF16, tag="tr")
                for cc in range(4):
                    nc.tensor.matmul(
                        tp[:, cc * P : cc * P + P], yc[:, cc * P : cc * P + P],
                        id_sb[:], is_transpose=True,
                        start=(cc == 0), stop=True, skip_group_check=True)
                nc.vector.tensor_copy(ytp[:], tp[:, 0:QT])
                ytp_t[p] = ytp

            def emit_rs():
                # ReduceScatter: core pair sums ar_in; even core keeps rows
                # [0:1024), odd core rows [1024:2048) -> host concatenates.
                # (Collectives cannot write IO tensors, so bounce rs_buf ->
                # SBUF -> outpart, copies spread over engine DMA queues.)
                nc.gpsimd.collective_compute(
                    "ReduceScatter",
                    mybir.AluOpType.add,
                    replica_groups=[[0, 1], [2, 3], [4, 5], [6, 7]],
                    ins=[ar_in[:].opt()],
                    outs=[rs_buf[:].opt()],
                )
                engs = [nc.sync, nc.scalar, nc.gpsimd]
                for blk in range(8):
                    oc = obpool.tile([P, D], F32, tag="oc", name=f"oc{blk}")
                    e = engs[blk % len(engs)]
                    e.dma_start(oc[:], rs_buf[blk * P : blk * P + P, :])
                    e.dma_start(outpart[blk * P : blk * P + P, :], oc[:])

            def emit_proj(i):
                for ss in range(4):
                    row = i * QT + ss * P
                    if proj_bias:
                        ps = prpool.tile([P, D], F32, tag="prjw")
                        for half in range(2):
                            for p in range(NP):
                                nc.tensor.matmul(
                                    ps[:, half * DL : half * DL + DL],
                                    lhsT=ytp_t[p][:, ss * P : ss * P + P],
                                    rhs=wpp[p][:, half * DL : half * DL + DL],
                                    start=(p == 0), stop=(p == NP - 1),
                                )
                        ob = obpool.tile([P, D], F32, tag="ob")
                        nc.vector.tensor_add(ob[:], ps[:], bp_sb[:])
                        nc.sync.dma_start(ar_in[row : row + P, :], ob[:])
                    else:
                        ob = obpool.tile([P, D], F32, tag="ob")
                        for half in range(2):
                            ps = prpool.tile([P, 512], F32, tag="prj")
                            for p in range(NP):
                                nc.tensor.matmul(
                                    ps[:, 0:DL],
                                    lhsT=ytp_t[p][:, ss * P : ss * P + P],
                                    rhs=wpp[p][:, half * DL : half * DL + DL],
                                    start=(p == 0), stop=(p == NP - 1),
                                )
                            nc.vector.tensor_copy(
                                ob[:, half * DL : half * DL + DL], ps[:, 0:DL])
                        nc.sync.dma_start(ar_in[row : row + P, :], ob[:])

            # ---- main schedule ----
            for m in range(2 * NP):
                emit_qk(m)
            for s in range(4):
                emit_v(s)
            for i in range(NI):
                for p in range(NP):
                    emit_attn(i, p)
                if i + 1 < NI:
                    for s in range(4 * (i + 1), 4 * (i + 2)):
                        emit_v(s)
                emit_proj(i)
            emit_rs()

    _legalize_waits(nc)
    return nc


_NC_CACHE = {}


def _get_nc(qk_bias=False, proj_bias=False):
    key = (qk_bias, proj_bias)
    if key not in _NC_CACHE:
        _NC_CACHE[key] = _build(qk_bias, proj_bias)
    return _NC_CACHE[key]


def _prep_inputs(x, W_attn, b_attn, W_proj, b_proj):
    bf = ml_dtypes.bfloat16
    f8 = ml_dtypes.float8_e4m3
    x = np.asarray(x, np.float32)
    W_attn = np.asarray(W_attn, np.float32)
    b_attn = np.asarray(b_attn, np.float32)
    W_proj = np.asarray(W_proj, np.float32)
    b_proj = np.asarray(b_proj, np.float32)

    k_idx = np.arange(P)[:, None]
    q_idx = np.arange(P)[None, :]
    mtri = np.where(q_idx >= k_idx, 0.0, MASKC).astype(bf)
    idn = np.eye(P).astype(bf)

    Wq = W_attn[:, 0:D]
    Wk = W_attn[:, D : 2 * D]
    Wv = W_attn[:, 2 * D :]

    in_maps = []
    meta = {}
    for c in range(N_CORES):
        b, g = divmod(c, 2)
        cols = slice(DL * g, DL * g + DL)
        xb = x[b]                                   # [S, D]
        # fp8 x in d-pair layout [3*128, 2*S]
        xq8 = np.empty((3 * P, 2 * S), f8)
        for t in range(NDP):
            for u in range(2):
                xq8[t * P : (t + 1) * P, u * S : (u + 1) * S] = (
                    xb[:, 256 * t + P * u : 256 * t + P * u + P].T.astype(f8))
        xkb = np.ascontiguousarray(xb.T).astype(bf)
        # fp8 W_attn q,k (scaled) in d-pair layout [3*128, 2*768]
        wa_qk = np.concatenate([Wq[:, cols], Wk[:, cols]], axis=1) * SA  # [D, 768]
        wq8 = np.empty((3 * P, 2 * 2 * DL), f8)
        for t in range(NDP):
            for u in range(2):
                wq8[t * P : (t + 1) * P, u * 2 * DL : (u + 1) * 2 * DL] = (
                    wa_qk[256 * t + P * u : 256 * t + P * u + P, :].astype(f8))
        wvb = np.ascontiguousarray(Wv[:, cols]).astype(bf)
        wpb = np.ascontiguousarray(W_proj[cols, :]).astype(bf)
        # qk bias (scaled): ba8[p, m] = SA * b[col m*128+p]
        ba_qk = np.concatenate([b_attn[0:D][cols], b_attn[D : 2 * D][cols]]) * SA
        ba8 = np.zeros((P, 8), np.float32)
        ba8[:, :6] = ba_qk.reshape(6, P).T
        # b_proj (+ v-bias folded) added once per row: only group 0 carries it
        bv = b_attn[2 * D :][cols]
        bp_eff = bv @ W_proj[cols, :] + (b_proj if g == 0 else 0.0)
        bpb = np.ascontiguousarray(
            np.broadcast_to(bp_eff.astype(np.float32), (P, D)))
        in_maps.append({
            "xq8": xq8, "xkb": xkb, "wq8": wq8, "wvb": wvb, "wpb": wpb,
            "ba8": ba8, "bpb": bpb, "mtri": mtri, "idn": idn,
        })
        meta.setdefault("qk_bias", bool(np.any(ba_qk != 0.0)))
        meta["proj_bias"] = meta.get("proj_bias", False) or bool(
            np.any(bp_eff != 0.0))
    return in_maps, meta


def kernel(x, W_attn, b_attn, W_proj, b_proj):
    in_maps, meta = _prep_inputs(x, W_attn, b_attn, W_proj, b_proj)
    nc = _get_nc(meta["qk_bias"], meta["proj_bias"])
    res = run_bass_kernel_spmd(nc, in_maps, list(range(N_CORES)))
    out = np.stack([
        np.concatenate(
            [res.results[2 * b]["outpart"], res.results[2 * b + 1]["outpart"]])
        for b in range(B)
    ])
    return out.astype(np.float32)


# revision 15
# speedup vs baseline: 1.1968x; 1.0521x over previous
"""Causal multi-head attention block (GPT-style) for Trainium2, 8 NeuronCores.

Problem: x[4,2048,768] -> qkv = x@W_attn+b_attn -> 12-head causal attention
         -> y@W_proj+b_proj -> out[4,2048,768]   (fp32 I/O)

Sharding: 4 batches x 2 head-groups (6 heads each); core c = 2*b + g handles
batch b, heads 6g..6g+5. c_proj row-sharded; AllReduce(add) over core pairs.

v2 kernel — fp8 DoubleRow (DR) matmuls everywhere the error budget allows:
  1. Q,K projection: fp8e4 DR over d-chunk pairs (contraction 256/instr,
     0.5 cyc/row). W_attn[q,k] scaled by SA=16 on host so fp8 sees ~N(0,0.3).
     PSUM -> (DVE +bias, fp8 out) qraw[128,S] -> DMA relayout to [32,2,S]
     per head so scores can run DR with K=32x2=64 (exact head_dim).
  2. V projection: bf16 (V feeds y almost linearly -> needs > fp8 accuracy),
     then split v = v_hi + v_lo (both fp8e4); the AV matmul consumes
     [v_hi|v_lo] as the two DR k-tiles with the SAME pt tile (stride-0 dim)
     => exact-to-fp8-residual V at DR speed.
  3. Scores S^T[k,q] per 128-k block via one DR instr per head (K=32x2).
     Causal diagonal: 128-wide staircase chunks get a shared tril mask
     ([128,128], 0/-3e5) ADDED in PSUM by DVE before exp (no post-mask).
  4. exp on ACT (the bottleneck engine): wide [128,<=1024] calls, fp8 out.
  5. AV transposed: out y_u[q-part, 65] per (q-128-chunk, head, j):
     lhsT = pt (stationary, stride-0 doubled), rhs = [v_hi|v_lo]. The ones
     column of v_hi makes col 64 the softmax denominator n[q] -- a
     per-partition scalar: normalize = DVE recip[128,4] + one strided mul.
  6. y chunks [128 q, 128 hd] -> PE transpose (bf16) -> y^T for proj.
  7. proj: bf16, contraction over the core's 3 pair-tiles, PSUM->DRAM DMA
     (b_proj+b_v@W_proj folded on host; DVE add only if nonzero).
  8. AllReduce(add) in 2 chunks: rows [0:1024] mid-kernel (hidden),
     [1024:2048] as the tail.

The walrus build allows only one sync-wait per instruction; legalize_waits
hoists extras onto single-wait NOPs.
"""
import numpy as np
import ml_dtypes

import concourse.bass as bass
import concourse.tile as tile
from concourse import mybir
from concourse.bass_utils import run_bass_kernel_spmd
from concourse import mybir as mb

BF16 = mybir.dt.bfloat16
F8 = mybir.dt.float8e4
F32 = mybir.dt.float32
DR = mybir.MatmulPerfMode.DoubleRow

B, S, D = 4, 2048, 768
H, HD = 12, 64
G = 2                 # head groups
HL = H // G           # heads per core = 6
DL = HL * HD          # local head dims = 384
NP = HL // 2          # head pairs per core = 3
P = 128
QT = 512              # q tile
NI = S // QT          # 4 q tiles
NS = S // P           # 16 k/s tiles
NDT = D // P          # 6 d tiles
NDP = NDT // 2        # 3 d pair tiles
N_CORES = 8
SA = 16.0             # host scale on W_attn[q,k] before fp8
ESC = 0.125 / (SA * SA)   # exp scale undoing SA^2 and 1/sqrt(hd)
MASKC = -3.0e5
# diag remainder layout inside scR (bank-crossing-free): mi -> col offset
REM_OFF = {0: 0, 1: 512, 2: 384}
REM_W = {0: 384, 1: 256, 2: 128}


def _legalize_waits(nc):
    n_split = 0
    for f in nc.m.functions:
        for bb in f.blocks:
            insts = list(bb.instructions)
            out = []
            changed = False
            for inst in insts:
                si = inst.sync_info
                if si is not None:
                    waits = list(si.on_wait)
                    if len(waits) > 1:
                        for w in waits[:-1]:
                            nop = mb.InstNoOp(name=f"I-wsplit-{nc.next_id()}", ins=[], outs=[])
                            nop.engine = inst.engine
                            nop.sync_info = mb.SyncInfo(on_wait=[w], on_update=[])
                            out.append(nop)
                            n_split += 1
                        inst.sync_info = mb.SyncInfo(on_wait=[waits[-1]], on_update=list(si.on_update))
                        changed = True
                out.append(inst)
            if changed:
                bb.instructions = out
    return n_split


def _build(qk_bias: bool, proj_bias: bool):
    nc = bass.Bass("TRN2", target_bir_lowering=False, debug=False, num_devices=N_CORES)

    xq8 = nc.dram_tensor("xq8", [3 * P, 2 * S], F8, kind="ExternalInput").ap()
    xkb = nc.dram_tensor("xkb", [D, S], BF16, kind="ExternalInput").ap()
    wq8 = nc.dram_tensor("wq8", [3 * P, 2 * 2 * DL], F8, kind="ExternalInput").ap()
    wvb = nc.dram_tensor("wvb", [D, DL], BF16, kind="ExternalInput").ap()
    wpb = nc.dram_tensor("wpb", [DL, D], BF16, kind="ExternalInput").ap()
    ba8 = nc.dram_tensor("ba8", [P, 8], F32, kind="ExternalInput").ap()
    bpb = nc.dram_tensor("bpb", [P, D], F32, kind="ExternalInput").ap()
    mtri = nc.dram_tensor("mtri", [P, P], BF16, kind="ExternalInput").ap()
    idn = nc.dram_tensor("idn", [P, P], BF16, kind="ExternalInput").ap()
    outpart = nc.dram_tensor("outpart", [S // 2, D], F32, kind="ExternalOutput").ap()
    ar_in = nc.dram_tensor("ar_in", [S, D], F32).ap()
    rs_buf = nc.dram_tensor("rs_buf", [S // 2, D], F32).ap()

    with tile.TileContext(nc) as tc:
        with (
            tc.tile_pool(name="wgt", bufs=1) as wpool,
            tc.tile_pool(name="qk", bufs=1) as qkpool,
            tc.tile_pool(name="vpr", bufs=1) as vpool,
            tc.tile_pool(name="pt", bufs=6) as ptpool,
            tc.tile_pool(name="yc", bufs=2) as ycpool,
            tc.tile_pool(name="ytp", bufs=2) as ytpool,
            tc.tile_pool(name="nrm", bufs=2) as nrmpool,
            tc.tile_pool(name="ob", bufs=3) as obpool,
            tc.tile_pool(name="scp", bufs=2, space="PSUM") as scpool,
            tc.tile_pool(name="avp", bufs=1, space="PSUM") as avpool,
            tc.tile_pool(name="trp", bufs=1, space="PSUM") as trpool,
            tc.tile_pool(name="prj", bufs=1, space="PSUM") as prpool,
        ):
            # ---- phase 0: load weights/constants ----
            # sync queue: only the QK-critical tensors, so scores start early;
            # everything else rides the scalar/gpsimd DMA queues in parallel
            wq = []   # fp8 wa-qk d-pair tiles [128, 2, 768]
            for t in range(NDP):
                wt = wpool.tile([P, 2 * 2 * DL], F8, tag=f"wq{t}")
                nc.sync.dma_start(wt[:], wq8[bass.ts(t, P), :])
                wq.append(wt)
            xq = []   # fp8 x d-pair tiles [128, 2, S]
            for t in range(NDP):
                xt = wpool.tile([P, 2 * S], F8, tag=f"xq{t}")
                nc.sync.dma_start(xt[:, 0:S], xq8[bass.ts(t, P), 0:S])
                nc.sync.dma_start(xt[:, S:], xq8[bass.ts(t, P), S:])
                xq.append(xt)
            xk = []   # bf16 x^T tiles [128, S] (V stationary)
            for t in range(NDT):
                xt = wpool.tile([P, S], BF16, tag=f"xk{t}")
                nc.scalar.dma_start(xt[:, 0 : S // 2], xkb[bass.ts(t, P), 0 : S // 2])
                nc.scalar.dma_start(xt[:, S // 2 :], xkb[bass.ts(t, P), S // 2 :])
                xk.append(xt)
            wv = []
            for t in range(NDT):
                wt = wpool.tile([P, DL], BF16, tag=f"wv{t}")
                nc.gpsimd.dma_start(wt[:], wvb[bass.ts(t, P), :])
                wv.append(wt)
            wpp = []
            for p in range(NP):
                wt = wpool.tile([P, D], BF16, tag=f"wp{p}")
                nc.gpsimd.dma_start(wt[:], wpb[bass.ts(p, P), :])
                wpp.append(wt)
            ba_sb = wpool.tile([P, 8], F32, tag="ba")
            if qk_bias:
                nc.gpsimd.dma_start(ba_sb[:], ba8[:])
            bp_sb = wpool.tile([P, D], F32, tag="bp")
            if proj_bias:
                nc.gpsimd.dma_start(bp_sb[:], bpb[:])
            mt_sb = wpool.tile([P, P], BF16, tag="mtri")
            nc.gpsimd.dma_start(mt_sb[:], mtri[:])
            id_sb = wpool.tile([P, P], BF16, tag="idn")
            nc.gpsimd.dma_start(id_sb[:], idn[:])
            # prewarm ScalarE's exp table
            warm_sb = wpool.tile([1, 2], F32, tag="warm")
            nc.vector.memset(warm_sb[:], 0.0)
            nc.scalar.activation(warm_sb[:, 1:2], warm_sb[:, 0:1],
                                 mybir.ActivationFunctionType.Exp)

            # ---- phase 1: Q,K -> fp8 [32,2,S]-layout tiles ----
            # qraw m-tiles 0..2 = Q pairs, 3..5 = K pairs, [128, S] fp8
            # (partitions: head A dims 0:64, head B dims 64:128)
            qt8 = [qkpool.tile([64, 2 * S], F8, tag=f"qt8{p}", name=f"qt8{p}") for p in range(NP)]
            kt8 = [qkpool.tile([64, 2 * S], F8, tag=f"kt8{p}", name=f"kt8{p}") for p in range(NP)]

            def emit_qk(m):
                qraw = qkpool.tile([P, S], F8, tag=f"qraw{m}")
                for n in range(S // QT):
                    ps = scpool.tile([P, 1024], F32, tag="sc")
                    for t in range(NDP):
                        nc.tensor.matmul(
                            ps[:, 0:QT],
                            lhsT=wq[t][:].rearrange("p (u m) -> p u m", u=2)[
                                :, :, bass.ts(m, P)],
                            rhs=xq[t][:].rearrange("p (u s) -> p u s", u=2)[
                                :, :, bass.ts(n, QT)],
                            start=(t == 0), stop=(t == NDP - 1), perf_mode=DR,
                        )
                    if qk_bias:
                        nc.vector.tensor_scalar_add(
                            qraw[:, bass.ts(n, QT)], ps[:, 0:QT], ba_sb[:, m : m + 1])
                    else:
                        nc.vector.tensor_copy(qraw[:, bass.ts(n, QT)], ps[:, 0:QT])
                # relayout [128, S] -> [32, 2, S] per head (A rows 0:32, B 32:64)
                dst = qt8[m] if m < NP else kt8[m - NP]
                engs = [nc.sync, nc.scalar, nc.gpsimd]
                for h in range(2):
                    for u in range(2):
                        engs[(2 * h + u + m) % 3].dma_start(
                            dst[32 * h : 32 * h + 32, u * S : (u + 1) * S],
                            qraw[64 * h + 32 * u : 64 * h + 32 * u + 32, :],
                        )

            # V s-tiles: [128, 780] fp8 = [hi 6x65 | lo 6x65], col 64-of-65:
            # hi=1 (softmax denominator via ones trick), lo=0
            v_t = [None] * NS

            def emit_v(s):
                ps = prpool.tile([P, 512], F32, tag="prj")
                for t in range(NDT):
                    nc.tensor.matmul(
                        ps[:, 0:DL],
                        lhsT=xk[t][:, bass.ts(s, P)],
                        rhs=wv[t][:],
                        start=(t == 0), stop=(t == NDT - 1),
                    )
                vt = vpool.tile([P, HL * 65], BF16, tag=f"v{s}")
                vt3 = vt[:].rearrange("p (h x) -> p h x", h=HL)
                ps3 = ps[:, 0:DL].rearrange("p (h x) -> p h x", h=HL)
                nc.vector.tensor_copy(vt3[:, :, 0:HD], ps3)
                nc.vector.memset(vt3[:, :, HD : HD + 1], 1.0)
                v_t[s] = vt

            ytp_t = [None] * NP

            def q_ap(p, h, q0, n):
                # rhs AP [32, 2, n] at q offset q0 for head h of pair p
                return qt8[p][32 * h : 32 * h + 32, :].rearrange(
                    "p (u s) -> p u s", u=2)[:, :, q0 : q0 + n]

            def k_ap(p, h, j):
                return kt8[p][32 * h : 32 * h + 32, :].rearrange(
                    "p (u s) -> p u s", u=2)[:, :, bass.ts(j, P)]

            def emit_attn(i, p):
                av = avpool.tile([P, 1024], F32, tag="av")
                av_started = [False, False]

                def av_mm(h, cc, j, pt_ap, stop):
                    # start=True zero-fills the whole PSUM bank, so exactly
                    # the first matmul touching each head's bank carries it
                    hh = 2 * p + h
                    nc.tensor.matmul(
                        av[:, 512 * h + cc * 65 : 512 * h + cc * 65 + 65],
                        lhsT=pt_ap,
                        rhs=v_t[j][:, 65 * hh : 65 * hh + 65],
                        start=not av_started[h], stop=stop,
                        skip_group_check=True,
                    )
                    av_started[h] = True

                # off-diagonal full groups (pairs of j blocks)
                for g in range(2 * i):
                    j0, j1 = 2 * g, 2 * g + 1
                    pts = []
                    for h in range(2):
                        sc = scpool.tile([P, 1024], F32, tag="sc")
                        for jj, off in ((j0, 0), (j1, QT)):
                            nc.tensor.matmul(
                                sc[:, off : off + QT],
                                lhsT=k_ap(p, h, jj),
                                rhs=q_ap(p, h, i * QT, QT),
                                start=True, stop=True, perf_mode=DR,
                            )
                        pt = ptpool.tile([P, 1024], BF16, tag="pt")
                        nc.scalar.activation(pt[:], sc[:],
                                             mybir.ActivationFunctionType.Exp,
                                             scale=ESC)
                        pts.append(pt)
                    for h in range(2):
                        for cc in range(4):
                            for jj, off in ((j0, 0), (j1, QT)):
                                av_mm(h, cc, jj,
                                      pts[h][:, off + cc * P : off + cc * P + P],
                                      stop=False)

                # diagonal: 8 staircase chunks (A 0:512 | B 512:1024) + mask
                scd = scpool.tile([P, 1024], F32, tag="sc")
                for h in range(2):
                    for mi in range(4):
                        nc.tensor.matmul(
                            scd[:, h * QT + mi * P : h * QT + mi * P + P],
                            lhsT=k_ap(p, h, 4 * i + mi),
                            rhs=q_ap(p, h, i * QT + mi * P, P),
                            start=(mi == 0), stop=False, perf_mode=DR,
                            skip_group_check=True,
                        )
                for h in range(2):
                    for mi in range(4):
                        # += mtri via PE: idn^T @ mtri = mtri (keeps the mask
                        # off DVE so exp never waits on the vector queue)
                        nc.tensor.matmul(
                            scd[:, h * QT + mi * P : h * QT + mi * P + P],
                            lhsT=id_sb[:], rhs=mt_sb[:],
                            start=False, stop=True, skip_group_check=True,
                        )
                ptD = ptpool.tile([P, 1024], BF16, tag="pt")
                nc.scalar.activation(ptD[:], scd[:],
                                     mybir.ActivationFunctionType.Exp, scale=ESC)
                # diagonal remainders (mask-free): mi0@0 w384, mi2@384 w128,
                # mi1@512 w256  (bank-crossing-free packing)
                ptR = []
                for h in range(2):
                    scr = scpool.tile([P, 1024], F32, tag="sc")
                    for mi in range(3):
                        w = REM_W[mi]
                        off = REM_OFF[mi]
                        nc.tensor.matmul(
                            scr[:, off : off + w],
                            lhsT=k_ap(p, h, 4 * i + mi),
                            rhs=q_ap(p, h, i * QT + mi * P + P, w),
                            start=(mi < 2), stop=True, perf_mode=DR,
                            skip_group_check=True,
                        )
                    ptr = ptpool.tile([P, 1024], BF16, tag="pt")
                    nc.scalar.activation(ptr[:, 0:768], scr[:, 0:768],
                                         mybir.ActivationFunctionType.Exp,
                                         scale=ESC)
                    ptR.append(ptr)
                for h in range(2):
                    for cc in range(4):
                        for mi in range(cc):
                            av_mm(h, cc, 4 * i + mi,
                                  ptR[h][:, REM_OFF[mi] + (cc - mi - 1) * P :
                                         REM_OFF[mi] + (cc - mi) * P],
                                  stop=False)
                        av_mm(h, cc, 4 * i + cc,
                              ptD[:, h * QT + cc * P : h * QT + cc * P + P],
                              stop=True)

                # normalize (per-partition scalar) + build y chunks
                rc = nrmpool.tile([P, 8], F32, tag="rc")
                yc = ycpool.tile([P, QT], BF16, tag="yc")
                yc4 = yc[:].rearrange("p (c n) -> p c n", c=4)
                for h in range(2):
                    av4 = av[:, 512 * h : 512 * h + 260].rearrange(
                        "p (c n) -> p c n", c=4)
                    nc.vector.reciprocal(
                        rc[:, 4 * h : 4 * h + 4].unsqueeze(2), av4[:, :, 64:65])
                    nc.vector.tensor_mul(
                        yc4[:, :, 64 * h : 64 * h + 64],
                        av4[:, :, 0:64],
                        rc[:, 4 * h : 4 * h + 4].unsqueeze(2).broadcast_to((P, 4, 64)),
                    )
                ytp = ytpool.tile([P, QT], BF16, tag=f"ytp{p}")
                tp = trpool.tile([P, 1024], BF16, tag="tr")
                for cc in range(4):
                    nc.tensor.matmul(
                        tp[:, cc * P : cc * P + P], yc[:, cc * P : cc * P + P],
                        id_sb[:], is_transpose=True,
                        start=(cc == 0), stop=True, skip_group_check=True)
                nc.vector.tensor_copy(ytp[:], tp[:, 0:QT])
                ytp_t[p] = ytp

            def emit_rs():
                # ReduceScatter: core pair sums ar_in; even core keeps rows
                # [0:1024), odd core rows [1024:2048) -> host concatenates.
                # (Collectives cannot write IO tensors, so bounce rs_buf ->
                # SBUF -> outpart, copies spread over engine DMA queues.)
                nc.gpsimd.collective_compute(
                    "ReduceScatter",
                    mybir.AluOpType.add,
                    replica_groups=[[0, 1], [2, 3], [4, 5], [6, 7]],
                    ins=[ar_in[:].opt()],
                    outs=[rs_buf[:].opt()],
                )
                engs = [nc.sync, nc.scalar, nc.gpsimd]
                for blk in range(4):
                    oc = obpool.tile([P, 2 * D], F32, tag="oc2", name=f"oc{blk}")
                    e = engs[blk % len(engs)]
                    src = rs_buf[blk * 256 : blk * 256 + 256, :].rearrange(
                        "(c p) d -> p c d", c=2)
                    dst = outpart[blk * 256 : blk * 256 + 256, :].rearrange(
                        "(c p) d -> p c d", c=2)
                    e.dma_start(oc[:].rearrange("p (c d) -> p c d", c=2), src)
                    e.dma_start(dst, oc[:].rearrange("p (c d) -> p c d", c=2))

            def emit_proj(i):
                for ss in range(4):
                    row = i * QT + ss * P
                    if proj_bias:
                        ps = prpool.tile([P, D], F32, tag="prjw")
                        for half in range(2):
                            for p in range(NP):
                                nc.tensor.matmul(
                                    ps[:, half * DL : half * DL + DL],
                                    lhsT=ytp_t[p][:, ss * P : ss * P + P],
                                    rhs=wpp[p][:, half * DL : half * DL + DL],
                                    start=(p == 0), stop=(p == NP - 1),
                                )
                        ob = obpool.tile([P, D], F32, tag="ob")
                        nc.vector.tensor_add(ob[:], ps[:], bp_sb[:])
                        nc.sync.dma_start(ar_in[row : row + P, :], ob[:])
                    else:
                        ob = obpool.tile([P, D], F32, tag="ob")
                        for half in range(2):
                            ps = prpool.tile([P, 512], F32, tag="prj")
                            for p in range(NP):
                                nc.tensor.matmul(
                                    ps[:, 0:DL],
                                    lhsT=ytp_t[p][:, ss * P : ss * P + P],
                                    rhs=wpp[p][:, half * DL : half * DL + DL],
                                    start=(p == 0), stop=(p == NP - 1),
                                )
                            nc.vector.tensor_copy(
                                ob[:, half * DL : half * DL + DL], ps[:, 0:DL])
                        nc.sync.dma_start(ar_in[row : row + P, :], ob[:])

            # ---- main schedule ----
            for m in range(2 * NP):
                emit_qk(m)
            for s in range(4):
                emit_v(s)
            for i in range(NI):
                for p in range(NP):
                    emit_attn(i, p)
                if i + 1 < NI:
                    for s in range(4 * (i + 1), 4 * (i + 2)):
                        emit_v(s)
                emit_proj(i)
            emit_rs()

    _legalize_waits(nc)
    return nc


_NC_CACHE = {}


def _get_nc(qk_bias=False, proj_bias=False):
    key = (qk_bias, proj_bias)
    if key not in _NC_CACHE:
        _NC_CACHE[key] = _build(qk_bias, proj_bias)
    return _NC_CACHE[key]


def _prep_inputs(x, W_attn, b_attn, W_proj, b_proj):
    bf = ml_dtypes.bfloat16
    f8 = ml_dtypes.float8_e4m3
    x = np.asarray(x, np.float32)
    W_attn = np.asarray(W_attn, np.float32)
    b_attn = np.asarray(b_attn, np.float32)
    W_proj = np.asarray(W_proj, np.float32)
    b_proj = np.asarray(b_proj, np.float32)

    k_idx = np.arange(P)[:, None]
    q_idx = np.arange(P)[None, :]
    mtri = np.where(q_idx >= k_idx, 0.0, MASKC).astype(bf)
    idn = np.eye(P).astype(bf)

    Wq = W_attn[:, 0:D]
    Wk = W_attn[:, D : 2 * D]
    Wv = W_attn[:, 2 * D :]

    in_maps = []
    meta = {}
    for c in range(N_CORES):
        b, g = divmod(c, 2)
        cols = slice(DL * g, DL * g + DL)
        xb = x[b]                                   # [S, D]
        # fp8 x in d-pair layout [3*128, 2*S]
        xq8 = np.empty((3 * P, 2 * S), f8)
        for t in range(NDP):
            for u in range(2):
                xq8[t * P : (t + 1) * P, u * S : (u + 1) * S] = (
                    xb[:, 256 * t + P * u : 256 * t + P * u + P].T.astype(f8))
        xkb = np.ascontiguousarray(xb.T).astype(bf)
        # fp8 W_attn q,k (scaled) in d-pair layout [3*128, 2*768]
        wa_qk = np.concatenate([Wq[:, cols], Wk[:, cols]], axis=1) * SA  # [D, 768]
        wq8 = np.empty((3 * P, 2 * 2 * DL), f8)
        for t in range(NDP):
            for u in range(2):
                wq8[t * P : (t + 1) * P, u * 2 * DL : (u + 1) * 2 * DL] = (
                    wa_qk[256 * t + P * u : 256 * t + P * u + P, :].astype(f8))
        wvb = np.ascontiguousarray(Wv[:, cols]).astype(bf)
        wpb = np.ascontiguousarray(W_proj[cols, :]).astype(bf)
        # qk bias (scaled): ba8[p, m] = SA * b[col m*128+p]
        ba_qk = np.concatenate([b_attn[0:D][cols], b_attn[D : 2 * D][cols]]) * SA
        ba8 = np.zeros((P, 8), np.float32)
        ba8[:, :6] = ba_qk.reshape(6, P).T
        # b_proj (+ v-bias folded) added once per row: only group 0 carries it
        bv = b_attn[2 * D :][cols]
        bp_eff = bv @ W_proj[cols, :] + (b_proj if g == 0 else 0.0)
        bpb = np.ascontiguousarray(
            np.broadcast_to(bp_eff.astype(np.float32), (P, D)))
        in_maps.append({
            "xq8": xq8, "xkb": xkb, "wq8": wq8, "wvb": wvb, "wpb": wpb,
            "ba8": ba8, "bpb": bpb, "mtri": mtri, "idn": idn,
        })
        meta.setdefault("qk_bias", bool(np.any(ba_qk != 0.0)))
        meta["proj_bias"] = meta.get("proj_bias", False) or bool(
            np.any(bp_eff != 0.0))
    return in_maps, meta


def kernel(x, W_attn, b_attn, W_proj, b_proj):
    in_maps, meta = _prep_inputs(x, W_attn, b_attn, W_proj, b_proj)
    nc = _get_nc(meta["qk_bias"], meta["proj_bias"])
    res = run_bass_kernel_spmd(nc, in_maps, list(range(N_CORES)))
    out = np.stack([
        np.concatenate(
            [res.results[2 * b]["outpart"], res.results[2 * b + 1]["outpart"]])
        for b in range(B)
    ])
    return out.astype(np.float32)


# revision 16
# speedup vs baseline: 1.1983x; 1.0012x over previous
"""Causal multi-head attention block (GPT-style) for Trainium2, 8 NeuronCores.

Problem: x[4,2048,768] -> qkv = x@W_attn+b_attn -> 12-head causal attention
         -> y@W_proj+b_proj -> out[4,2048,768]   (fp32 I/O)

Sharding: 4 batches x 2 head-groups (6 heads each); core c = 2*b + g handles
batch b, heads 6g..6g+5. c_proj row-sharded; AllReduce(add) over core pairs.

v2 kernel — fp8 DoubleRow (DR) matmuls everywhere the error budget allows:
  1. Q,K projection: fp8e4 DR over d-chunk pairs (contraction 256/instr,
     0.5 cyc/row). W_attn[q,k] scaled by SA=16 on host so fp8 sees ~N(0,0.3).
     PSUM -> (DVE +bias, fp8 out) qraw[128,S] -> DMA relayout to [32,2,S]
     per head so scores can run DR with K=32x2=64 (exact head_dim).
  2. V projection: bf16 (V feeds y almost linearly -> needs > fp8 accuracy),
     then split v = v_hi + v_lo (both fp8e4); the AV matmul consumes
     [v_hi|v_lo] as the two DR k-tiles with the SAME pt tile (stride-0 dim)
     => exact-to-fp8-residual V at DR speed.
  3. Scores S^T[k,q] per 128-k block via one DR instr per head (K=32x2).
     Causal diagonal: 128-wide staircase chunks get a shared tril mask
     ([128,128], 0/-3e5) ADDED in PSUM by DVE before exp (no post-mask).
  4. exp on ACT (the bottleneck engine): wide [128,<=1024] calls, fp8 out.
  5. AV transposed: out y_u[q-part, 65] per (q-128-chunk, head, j):
     lhsT = pt (stationary, stride-0 doubled), rhs = [v_hi|v_lo]. The ones
     column of v_hi makes col 64 the softmax denominator n[q] -- a
     per-partition scalar: normalize = DVE recip[128,4] + one strided mul.
  6. y chunks [128 q, 128 hd] -> PE transpose (bf16) -> y^T for proj.
  7. proj: bf16, contraction over the core's 3 pair-tiles, PSUM->DRAM DMA
     (b_proj+b_v@W_proj folded on host; DVE add only if nonzero).
  8. AllReduce(add) in 2 chunks: rows [0:1024] mid-kernel (hidden),
     [1024:2048] as the tail.

The walrus build allows only one sync-wait per instruction; legalize_waits
hoists extras onto single-wait NOPs.
"""
import numpy as np
import ml_dtypes

import concourse.bass as bass
import concourse.tile as tile
from concourse import mybir
from concourse.bass_utils import run_bass_kernel_spmd
from concourse import mybir as mb

BF16 = mybir.dt.bfloat16
F8 = mybir.dt.float8e4
F32 = mybir.dt.float32
DR = mybir.MatmulPerfMode.DoubleRow

B, S, D = 4, 2048, 768
H, HD = 12, 64
G = 2                 # head groups
HL = H // G           # heads per core = 6
DL = HL * HD          # local head dims = 384
NP = HL // 2          # head pairs per core = 3
P = 128
QT = 512              # q tile
NI = S // QT          # 4 q tiles
NS = S // P           # 16 k/s tiles
NDT = D // P          # 6 d tiles
NDP = NDT // 2        # 3 d pair tiles
N_CORES = 8
SA = 16.0             # host scale on W_attn[q,k] before fp8
ESC = 0.125 / (SA * SA)   # exp scale undoing SA^2 and 1/sqrt(hd)
MASKC = -3.0e5
# diag remainder layout inside scR (bank-crossing-free): mi -> col offset
REM_OFF = {0: 0, 1: 512, 2: 384}
REM_W = {0: 384, 1: 256, 2: 128}


def _legalize_waits(nc):
    n_split = 0
    for f in nc.m.functions:
        for bb in f.blocks:
            insts = list(bb.instructions)
            out = []
            changed = False
            for inst in insts:
                si = inst.sync_info
                if si is not None:
                    waits = list(si.on_wait)
                    if len(waits) > 1:
                        for w in waits[:-1]:
                            nop = mb.InstNoOp(name=f"I-wsplit-{nc.next_id()}", ins=[], outs=[])
                            nop.engine = inst.engine
                            nop.sync_info = mb.SyncInfo(on_wait=[w], on_update=[])
                            out.append(nop)
                            n_split += 1
                        inst.sync_info = mb.SyncInfo(on_wait=[waits[-1]], on_update=list(si.on_update))
                        changed = True
                out.append(inst)
            if changed:
                bb.instructions = out
    return n_split


def _build(qk_bias: bool, proj_bias: bool):
    nc = bass.Bass("TRN2", target_bir_lowering=False, debug=False, num_devices=N_CORES)

    xq8 = nc.dram_tensor("xq8", [3 * P, 2 * S], F8, kind="ExternalInput").ap()
    xkb = nc.dram_tensor("xkb", [D, S], BF16, kind="ExternalInput").ap()
    wq8 = nc.dram_tensor("wq8", [3 * P, 2 * 2 * DL], F8, kind="ExternalInput").ap()
    wvb = nc.dram_tensor("wvb", [D, DL], BF16, kind="ExternalInput").ap()
    wpb = nc.dram_tensor("wpb", [DL, D], BF16, kind="ExternalInput").ap()
    ba8 = nc.dram_tensor("ba8", [P, 8], F32, kind="ExternalInput").ap()
    bpb = nc.dram_tensor("bpb", [P, D], F32, kind="ExternalInput").ap()
    mtri = nc.dram_tensor("mtri", [P, P], BF16, kind="ExternalInput").ap()
    idn = nc.dram_tensor("idn", [P, P], BF16, kind="ExternalInput").ap()
    outpart = nc.dram_tensor("outpart", [S // 2, D], F32, kind="ExternalOutput").ap()
    ar_in = nc.dram_tensor("ar_in", [S, D], F32).ap()
    rs_buf = nc.dram_tensor("rs_buf", [S // 2, D], F32).ap()

    with tile.TileContext(nc) as tc:
        with (
            tc.tile_pool(name="wgt", bufs=1) as wpool,
            tc.tile_pool(name="qk", bufs=1) as qkpool,
            tc.tile_pool(name="vpr", bufs=1) as vpool,
            tc.tile_pool(name="pt", bufs=6) as ptpool,
            tc.tile_pool(name="yc", bufs=2) as ycpool,
            tc.tile_pool(name="ytp", bufs=2) as ytpool,
            tc.tile_pool(name="nrm", bufs=2) as nrmpool,
            tc.tile_pool(name="ob", bufs=3) as obpool,
            tc.tile_pool(name="scp", bufs=2, space="PSUM") as scpool,
            tc.tile_pool(name="avp", bufs=1, space="PSUM") as avpool,
            tc.tile_pool(name="trp", bufs=1, space="PSUM") as trpool,
            tc.tile_pool(name="prj", bufs=1, space="PSUM") as prpool,
        ):
            # ---- phase 0: load weights/constants ----
            # sync queue: only the QK-critical tensors, so scores start early;
            # everything else rides the scalar/gpsimd DMA queues in parallel
            wq = []   # fp8 wa-qk d-pair tiles [128, 2, 768]
            for t in range(NDP):
                wt = wpool.tile([P, 2 * 2 * DL], F8, tag=f"wq{t}")
                nc.sync.dma_start(wt[:], wq8[bass.ts(t, P), :])
                wq.append(wt)
            xq = []   # fp8 x d-pair tiles [128, 2, S]
            for t in range(NDP):
                xt = wpool.tile([P, 2 * S], F8, tag=f"xq{t}")
                nc.sync.dma_start(xt[:, 0:S], xq8[bass.ts(t, P), 0:S])
                nc.sync.dma_start(xt[:, S:], xq8[bass.ts(t, P), S:])
                xq.append(xt)
            xk = []   # bf16 x^T tiles [128, S] (V stationary)
            for t in range(NDT):
                xt = wpool.tile([P, S], BF16, tag=f"xk{t}")
                nc.gpsimd.dma_start(xt[:, 0 : S // 2], xkb[bass.ts(t, P), 0 : S // 2])
                nc.gpsimd.dma_start(xt[:, S // 2 :], xkb[bass.ts(t, P), S // 2 :])
                xk.append(xt)
            wv = []
            for t in range(NDT):
                wt = wpool.tile([P, DL], BF16, tag=f"wv{t}")
                nc.gpsimd.dma_start(wt[:], wvb[bass.ts(t, P), :])
                wv.append(wt)
            wpp = []
            for p in range(NP):
                wt = wpool.tile([P, D], BF16, tag=f"wp{p}")
                nc.gpsimd.dma_start(wt[:], wpb[bass.ts(p, P), :])
                wpp.append(wt)
            ba_sb = wpool.tile([P, 8], F32, tag="ba")
            if qk_bias:
                nc.gpsimd.dma_start(ba_sb[:], ba8[:])
            bp_sb = wpool.tile([P, D], F32, tag="bp")
            if proj_bias:
                nc.gpsimd.dma_start(bp_sb[:], bpb[:])
            mt_sb = wpool.tile([P, P], BF16, tag="mtri")
            nc.gpsimd.dma_start(mt_sb[:], mtri[:])
            id_sb = wpool.tile([P, P], BF16, tag="idn")
            nc.gpsimd.dma_start(id_sb[:], idn[:])
            # prewarm ScalarE's exp table
            warm_sb = wpool.tile([1, 2], F32, tag="warm")
            nc.vector.memset(warm_sb[:], 0.0)
            nc.scalar.activation(warm_sb[:, 1:2], warm_sb[:, 0:1],
                                 mybir.ActivationFunctionType.Exp)

            # ---- phase 1: Q,K -> fp8 [32,2,S]-layout tiles ----
            # qraw m-tiles 0..2 = Q pairs, 3..5 = K pairs, [128, S] fp8
            # (partitions: head A dims 0:64, head B dims 64:128)
            qt8 = [qkpool.tile([64, 2 * S], F8, tag=f"qt8{p}", name=f"qt8{p}") for p in range(NP)]
            kt8 = [qkpool.tile([64, 2 * S], F8, tag=f"kt8{p}", name=f"kt8{p}") for p in range(NP)]

            def emit_qk(m):
                qraw = qkpool.tile([P, S], F8, tag=f"qraw{m}")
                for n in range(S // QT):
                    ps = scpool.tile([P, 1024], F32, tag="sc")
                    for t in range(NDP):
                        nc.tensor.matmul(
                            ps[:, 0:QT],
                            lhsT=wq[t][:].rearrange("p (u m) -> p u m", u=2)[
                                :, :, bass.ts(m, P)],
                            rhs=xq[t][:].rearrange("p (u s) -> p u s", u=2)[
                                :, :, bass.ts(n, QT)],
                            start=(t == 0), stop=(t == NDP - 1), perf_mode=DR,
                        )
                    if qk_bias:
                        nc.vector.tensor_scalar_add(
                            qraw[:, bass.ts(n, QT)], ps[:, 0:QT], ba_sb[:, m : m + 1])
                    else:
                        nc.vector.tensor_copy(qraw[:, bass.ts(n, QT)], ps[:, 0:QT])
                # relayout [128, S] -> [32, 2, S] per head (A rows 0:32, B 32:64)
                dst = qt8[m] if m < NP else kt8[m - NP]
                engs = [nc.sync, nc.gpsimd]
                for h in range(2):
                    for u in range(2):
                        engs[(2 * h + u + m) % 2].dma_start(
                            dst[32 * h : 32 * h + 32, u * S : (u + 1) * S],
                            qraw[64 * h + 32 * u : 64 * h + 32 * u + 32, :],
                        )

            # V s-tiles: [128, 780] fp8 = [hi 6x65 | lo 6x65], col 64-of-65:
            # hi=1 (softmax denominator via ones trick), lo=0
            v_t = [None] * NS

            def emit_v(s):
                ps = prpool.tile([P, 512], F32, tag="prj")
                for t in range(NDT):
                    nc.tensor.matmul(
                        ps[:, 0:DL],
                        lhsT=xk[t][:, bass.ts(s, P)],
                        rhs=wv[t][:],
                        start=(t == 0), stop=(t == NDT - 1),
                    )
                vt = vpool.tile([P, HL * 65], BF16, tag=f"v{s}")
                vt3 = vt[:].rearrange("p (h x) -> p h x", h=HL)
                ps3 = ps[:, 0:DL].rearrange("p (h x) -> p h x", h=HL)
                nc.vector.tensor_copy(vt3[:, :, 0:HD], ps3)
                nc.vector.memset(vt3[:, :, HD : HD + 1], 1.0)
                v_t[s] = vt

            ytp_t = [None] * NP

            def q_ap(p, h, q0, n):
                # rhs AP [32, 2, n] at q offset q0 for head h of pair p
                return qt8[p][32 * h : 32 * h + 32, :].rearrange(
                    "p (u s) -> p u s", u=2)[:, :, q0 : q0 + n]

            def k_ap(p, h, j):
                return kt8[p][32 * h : 32 * h + 32, :].rearrange(
                    "p (u s) -> p u s", u=2)[:, :, bass.ts(j, P)]

            def emit_attn(i, p):
                av = avpool.tile([P, 1024], F32, tag="av")
                av_started = [False, False]

                def av_mm(h, cc, j, pt_ap, stop):
                    # start=True zero-fills the whole PSUM bank, so exactly
                    # the first matmul touching each head's bank carries it
                    hh = 2 * p + h
                    nc.tensor.matmul(
                        av[:, 512 * h + cc * 65 : 512 * h + cc * 65 + 65],
                        lhsT=pt_ap,
                        rhs=v_t[j][:, 65 * hh : 65 * hh + 65],
                        start=not av_started[h], stop=stop,
                        skip_group_check=True,
                    )
                    av_started[h] = True

                # off-diagonal full groups (pairs of j blocks)
                for g in range(2 * i):
                    j0, j1 = 2 * g, 2 * g + 1
                    pts = []
                    for h in range(2):
                        sc = scpool.tile([P, 1024], F32, tag="sc")
                        for jj, off in ((j0, 0), (j1, QT)):
                            nc.tensor.matmul(
                                sc[:, off : off + QT],
                                lhsT=k_ap(p, h, jj),
                                rhs=q_ap(p, h, i * QT, QT),
                                start=True, stop=True, perf_mode=DR,
                            )
                        pt = ptpool.tile([P, 1024], BF16, tag="pt")
                        nc.scalar.activation(pt[:], sc[:],
                                             mybir.ActivationFunctionType.Exp,
                                             scale=ESC)
                        pts.append(pt)
                    for h in range(2):
                        for cc in range(4):
                            for jj, off in ((j0, 0), (j1, QT)):
                                av_mm(h, cc, jj,
                                      pts[h][:, off + cc * P : off + cc * P + P],
                                      stop=False)

                # diagonal: 8 staircase chunks (A 0:512 | B 512:1024) + mask
                scd = scpool.tile([P, 1024], F32, tag="sc")
                for h in range(2):
                    for mi in range(4):
                        nc.tensor.matmul(
                            scd[:, h * QT + mi * P : h * QT + mi * P + P],
                            lhsT=k_ap(p, h, 4 * i + mi),
                            rhs=q_ap(p, h, i * QT + mi * P, P),
                            start=(mi == 0), stop=False, perf_mode=DR,
                            skip_group_check=True,
                        )
                for h in range(2):
                    for mi in range(4):
                        # += mtri via PE: idn^T @ mtri = mtri (keeps the mask
                        # off DVE so exp never waits on the vector queue)
                        nc.tensor.matmul(
                            scd[:, h * QT + mi * P : h * QT + mi * P + P],
                            lhsT=id_sb[:], rhs=mt_sb[:],
                            start=False, stop=True, skip_group_check=True,
                        )
                ptD = ptpool.tile([P, 1024], BF16, tag="pt")
                nc.scalar.activation(ptD[:], scd[:],
                                     mybir.ActivationFunctionType.Exp, scale=ESC)
                # diagonal remainders (mask-free): mi0@0 w384, mi2@384 w128,
                # mi1@512 w256  (bank-crossing-free packing)
                ptR = []
                for h in range(2):
                    scr = scpool.tile([P, 1024], F32, tag="sc")
                    for mi in range(3):
                        w = REM_W[mi]
                        off = REM_OFF[mi]
                        nc.tensor.matmul(
                            scr[:, off : off + w],
                            lhsT=k_ap(p, h, 4 * i + mi),
                            rhs=q_ap(p, h, i * QT + mi * P + P, w),
                            start=(mi < 2), stop=True, perf_mode=DR,
                            skip_group_check=True,
                        )
                    ptr = ptpool.tile([P, 1024], BF16, tag="pt")
                    nc.scalar.activation(ptr[:, 0:768], scr[:, 0:768],
                                         mybir.ActivationFunctionType.Exp,
                                         scale=ESC)
                    ptR.append(ptr)
                for h in range(2):
                    for cc in range(4):
                        for mi in range(cc):
                            av_mm(h, cc, 4 * i + mi,
                                  ptR[h][:, REM_OFF[mi] + (cc - mi - 1) * P :
                                         REM_OFF[mi] + (cc - mi) * P],
                                  stop=False)
                        av_mm(h, cc, 4 * i + cc,
                              ptD[:, h * QT + cc * P : h * QT + cc * P + P],
                              stop=True)

                # normalize (per-partition scalar) + build y chunks
                rc = nrmpool.tile([P, 8], F32, tag="rc")
                yc = ycpool.tile([P, QT], BF16, tag="yc")
                yc4 = yc[:].rearrange("p (c n) -> p c n", c=4)
                for h in range(2):
                    av4 = av[:, 512 * h : 512 * h + 260].rearrange(
                        "p (c n) -> p c n", c=4)
                    nc.vector.reciprocal(
                        rc[:, 4 * h : 4 * h + 4].unsqueeze(2), av4[:, :, 64:65])
                    nc.vector.tensor_mul(
                        yc4[:, :, 64 * h : 64 * h + 64],
                        av4[:, :, 0:64],
                        rc[:, 4 * h : 4 * h + 4].unsqueeze(2).broadcast_to((P, 4, 64)),
                    )
                ytp = ytpool.tile([P, QT], BF16, tag=f"ytp{p}")
                tp = trpool.tile([P, 1024], BF16, tag="tr")
                for cc in range(4):
                    nc.tensor.matmul(
                        tp[:, cc * P : cc * P + P], yc[:, cc * P : cc * P + P],
                        id_sb[:], is_transpose=True,
                        start=(cc == 0), stop=True, skip_group_check=True)
                nc.vector.tensor_copy(ytp[:], tp[:, 0:QT])
                ytp_t[p] = ytp

            def emit_rs():
                # ReduceScatter: core pair sums ar_in; even core keeps rows
                # [0:1024), odd core rows [1024:2048) -> host concatenates.
                # (Collectives cannot write IO tensors, so bounce rs_buf ->
                # SBUF -> outpart, copies spread over engine DMA queues.)
                nc.gpsimd.collective_compute(
                    "ReduceScatter",
                    mybir.AluOpType.add,
                    replica_groups=[[0, 1], [2, 3], [4, 5], [6, 7]],
                    ins=[ar_in[:].opt()],
                    outs=[rs_buf[:].opt()],
                )
                engs = [nc.sync, nc.gpsimd]
                for blk in range(4):
                    oc = obpool.tile([P, 2 * D], F32, tag="oc2", name=f"oc{blk}")
                    e = engs[blk % len(engs)]
                    src = rs_buf[blk * 256 : blk * 256 + 256, :].rearrange(
                        "(c p) d -> p c d", c=2)
                    dst = outpart[blk * 256 : blk * 256 + 256, :].rearrange(
                        "(c p) d -> p c d", c=2)
                    e.dma_start(oc[:].rearrange("p (c d) -> p c d", c=2), src)
                    e.dma_start(dst, oc[:].rearrange("p (c d) -> p c d", c=2))

            def emit_proj(i):
                for ss in range(4):
                    row = i * QT + ss * P
                    if proj_bias:
                        ps = prpool.tile([P, D], F32, tag="prjw")
                        for half in range(2):
                            for p in range(NP):
                                nc.tensor.matmul(
                                    ps[:, half * DL : half * DL + DL],
                                    lhsT=ytp_t[p][:, ss * P : ss * P + P],
                                    rhs=wpp[p][:, half * DL : half * DL + DL],
                                    start=(p == 0), stop=(p == NP - 1),
                                )
                        ob = obpool.tile([P, D], F32, tag="ob")
                        nc.vector.tensor_add(ob[:], ps[:], bp_sb[:])
                        nc.sync.dma_start(ar_in[row : row + P, :], ob[:])
                    else:
                        ob = obpool.tile([P, D], F32, tag="ob")
                        for half in range(2):
                            ps = prpool.tile([P, 512], F32, tag="prj")
                            for p in range(NP):
                                nc.tensor.matmul(
                                    ps[:, 0:DL],
                                    lhsT=ytp_t[p][:, ss * P : ss * P + P],
                                    rhs=wpp[p][:, half * DL : half * DL + DL],
                                    start=(p == 0), stop=(p == NP - 1),
                                )
                            nc.vector.tensor_copy(
                                ob[:, half * DL : half * DL + DL], ps[:, 0:DL])
                        nc.sync.dma_start(ar_in[row : row + P, :], ob[:])

            # ---- main schedule ----
            for m in (0, 3, 1, 4, 2, 5):
                emit_qk(m)
            for s in range(4):
                emit_v(s)
            for i in range(NI):
                for p in range(NP):
                    emit_attn(i, p)
                if i + 1 < NI:
                    for s in range(4 * (i + 1), 4 * (i + 2)):
                        emit_v(s)
                emit_proj(i)
            emit_rs()

    _legalize_waits(nc)
    return nc


_NC_CACHE = {}


def _get_nc(qk_bias=False, proj_bias=False):
    key = (qk_bias, proj_bias)
    if key not in _NC_CACHE:
        _NC_CACHE[key] = _build(qk_bias, proj_bias)
    return _NC_CACHE[key]


def _prep_inputs(x, W_attn, b_attn, W_proj, b_proj):
    bf = ml_dtypes.bfloat16
    f8 = ml_dtypes.float8_e4m3
    x = np.asarray(x, np.float32)
    W_attn = np.asarray(W_attn, np.float32)
    b_attn = np.asarray(b_attn, np.float32)
    W_proj = np.asarray(W_proj, np.float32)
    b_proj = np.asarray(b_proj, np.float32)

    k_idx = np.arange(P)[:, None]
    q_idx = np.arange(P)[None, :]
    mtri = np.where(q_idx >= k_idx, 0.0, MASKC).astype(bf)
    idn = np.eye(P).astype(bf)

    Wq = W_attn[:, 0:D]
    Wk = W_attn[:, D : 2 * D]
    Wv = W_attn[:, 2 * D :]

    in_maps = []
    meta = {}
    for c in range(N_CORES):
        b, g = divmod(c, 2)
        cols = slice(DL * g, DL * g + DL)
        xb = x[b]                                   # [S, D]
        # fp8 x in d-pair layout [3*128, 2*S]
        xq8 = np.empty((3 * P, 2 * S), f8)
        for t in range(NDP):
            for u in range(2):
                xq8[t * P : (t + 1) * P, u * S : (u + 1) * S] = (
                    xb[:, 256 * t + P * u : 256 * t + P * u + P].T.astype(f8))
        xkb = np.ascontiguousarray(xb.T).astype(bf)
        # fp8 W_attn q,k (scaled) in d-pair layout [3*128, 2*768]
        wa_qk = np.concatenate([Wq[:, cols], Wk[:, cols]], axis=1) * SA  # [D, 768]
        wq8 = np.empty((3 * P, 2 * 2 * DL), f8)
        for t in range(NDP):
            for u in range(2):
                wq8[t * P : (t + 1) * P, u * 2 * DL : (u + 1) * 2 * DL] = (
                    wa_qk[256 * t + P * u : 256 * t + P * u + P, :].astype(f8))
        wvb = np.ascontiguousarray(Wv[:, cols]).astype(bf)
        wpb = np.ascontiguousarray(W_proj[cols, :]).astype(bf)
        # qk bias (scaled): ba8[p, m] = SA * b[col m*128+p]
        ba_qk = np.concatenate([b_attn[0:D][cols], b_attn[D : 2 * D][cols]]) * SA
        ba8 = np.zeros((P, 8), np.float32)
        ba8[:, :6] = ba_qk.reshape(6, P).T
        # b_proj (+ v-bias folded) added once per row: only group 0 carries it
        bv = b_attn[2 * D :][cols]
        bp_eff = bv @ W_proj[cols, :] + (b_proj if g == 0 else 0.0)
        bpb = np.ascontiguousarray(
            np.broadcast_to(bp_eff.astype(np.float32), (P, D)))
        in_maps.append({
            "xq8": xq8, "xkb": xkb, "wq8": wq8, "wvb": wvb, "wpb": wpb,
            "ba8": ba8, "bpb": bpb, "mtri": mtri, "idn": idn,
        })
        meta.setdefault("qk_bias", bool(np.any(ba_qk != 0.0)))
        meta["proj_bias"] = meta.get("proj_bias", False) or bool(
            np.any(bp_eff != 0.0))
    return in_maps, meta


def kernel(x, W_attn, b_attn, W_proj, b_proj):
    in_maps, meta = _prep_inputs(x, W_attn, b_attn, W_proj, b_proj)
    nc = _get_nc(meta["qk_bias"], meta["proj_bias"])
    res = run_bass_kernel_spmd(nc, in_maps, list(range(N_CORES)))
    out = np.stack([
        np.concatenate(
            [res.results[2 * b]["outpart"], res.results[2 * b + 1]["outpart"]])
        for b in range(B)
    ])
    return out.astype(np.float32)


# revision 29
# speedup vs baseline: 1.2812x; 1.0692x over previous
"""Causal multi-head attention block (GPT-style) for Trainium2, 8 NeuronCores.

Problem: x[4,2048,768] -> qkv = x@W_attn+b_attn -> 12-head causal attention
         -> y@W_proj+b_proj -> out[4,2048,768]   (fp32 I/O)

Sharding: 4 batches x 2 head-groups (6 heads each); core c = 2*b + g handles
batch b, heads 6g..6g+5. c_proj row-sharded; AllReduce(add) over core pairs.

v2 kernel — fp8 DoubleRow (DR) matmuls everywhere the error budget allows:
  1. Q,K projection: fp8e4 DR over d-chunk pairs (contraction 256/instr,
     0.5 cyc/row). W_attn[q,k] scaled by SA=16 on host so fp8 sees ~N(0,0.3).
     PSUM -> (DVE +bias, fp8 out) qraw[128,S] -> DMA relayout to [32,2,S]
     per head so scores can run DR with K=32x2=64 (exact head_dim).
  2. V projection: bf16 (V feeds y almost linearly -> needs > fp8 accuracy),
     then split v = v_hi + v_lo (both fp8e4); the AV matmul consumes
     [v_hi|v_lo] as the two DR k-tiles with the SAME pt tile (stride-0 dim)
     => exact-to-fp8-residual V at DR speed.
  3. Scores S^T[k,q] per 128-k block via one DR instr per head (K=32x2).
     Causal diagonal: 128-wide staircase chunks get a shared tril mask
     ([128,128], 0/-3e5) ADDED in PSUM by DVE before exp (no post-mask).
  4. exp on ACT (the bottleneck engine): wide [128,<=1024] calls, fp8 out.
  5. AV transposed: out y_u[q-part, 65] per (q-128-chunk, head, j):
     lhsT = pt (stationary, stride-0 doubled), rhs = [v_hi|v_lo]. The ones
     column of v_hi makes col 64 the softmax denominator n[q] -- a
     per-partition scalar: normalize = DVE recip[128,4] + one strided mul.
  6. y chunks [128 q, 128 hd] -> PE transpose (bf16) -> y^T for proj.
  7. proj: bf16, contraction over the core's 3 pair-tiles, PSUM->DRAM DMA
     (b_proj+b_v@W_proj folded on host; DVE add only if nonzero).
  8. AllReduce(add) in 2 chunks: rows [0:1024] mid-kernel (hidden),
     [1024:2048] as the tail.

The walrus build allows only one sync-wait per instruction; legalize_waits
hoists extras onto single-wait NOPs.
"""
import numpy as np
import ml_dtypes

import concourse.bass as bass
import concourse.tile as tile
from concourse import mybir
from concourse.bass_utils import run_bass_kernel_spmd
from concourse import mybir as mb

BF16 = mybir.dt.bfloat16
F8 = mybir.dt.float8e4
F32 = mybir.dt.float32
DR = mybir.MatmulPerfMode.DoubleRow

B, S, D = 4, 2048, 768
H, HD = 12, 64
G = 2                 # head groups
HL = H // G           # heads per core = 6
DL = HL * HD          # local head dims = 384
NP = HL // 2          # head pairs per core = 3
P = 128
QT = 512              # q tile
NI = S // QT          # 4 q tiles
NS = S // P           # 16 k/s tiles
NDT = D // P          # 6 d tiles
NDP = NDT // 2        # 3 d pair tiles
N_CORES = 8
SA = 16.0             # host scale on W_attn[q,k] before fp8
ESC = 0.125 / (SA * SA)   # exp scale undoing SA^2 and 1/sqrt(hd)
MASKC = -3.0e5
# diag remainder layout inside scR (bank-crossing-free): mi -> col offset
REM_OFF = {0: 0, 1: 512, 2: 384}
REM_W = {0: 384, 1: 256, 2: 128}


def _legalize_waits(nc):
    n_split = 0
    for f in nc.m.functions:
        for bb in f.blocks:
            insts = list(bb.instructions)
            out = []
            changed = False
            for inst in insts:
                si = inst.sync_info
                if si is not None:
                    waits = list(si.on_wait)
                    if len(waits) > 1:
                        for w in waits[:-1]:
                            nop = mb.InstNoOp(name=f"I-wsplit-{nc.next_id()}", ins=[], outs=[])
                            nop.engine = inst.engine
                            nop.sync_info = mb.SyncInfo(on_wait=[w], on_update=[])
                            out.append(nop)
                            n_split += 1
                        inst.sync_info = mb.SyncInfo(on_wait=[waits[-1]], on_update=list(si.on_update))
                        changed = True
                out.append(inst)
            if changed:
                bb.instructions = out
    return n_split


def _build(qk_bias: bool, proj_bias: bool):
    nc = bass.Bass("TRN2", target_bir_lowering=False, debug=False, num_devices=N_CORES)

    xq8 = nc.dram_tensor("xq8", [3 * P, 2 * S], F8, kind="ExternalInput").ap()
    xkb = nc.dram_tensor("xkb", [D, S], BF16, kind="ExternalInput").ap()
    wq8 = nc.dram_tensor("wq8", [3 * P, 2 * 2 * DL], F8, kind="ExternalInput").ap()
    wvb = nc.dram_tensor("wvb", [D, DL], BF16, kind="ExternalInput").ap()
    wpb = nc.dram_tensor("wpb", [DL, D], BF16, kind="ExternalInput").ap()
    ba8 = nc.dram_tensor("ba8", [P, 8], F32, kind="ExternalInput").ap()
    bpb = nc.dram_tensor("bpb", [P, D], F32, kind="ExternalInput").ap()
    mtri = nc.dram_tensor("mtri", [P, P], BF16, kind="ExternalInput").ap()
    idn = nc.dram_tensor("idn", [P, P], BF16, kind="ExternalInput").ap()
    outpart = nc.dram_tensor("outpart", [S // 2, D], F32, kind="ExternalOutput").ap()
    ar_in = nc.dram_tensor("ar_in", [S, D], F32).ap()
    rs_buf = nc.dram_tensor("rs_buf", [S // 2, D], F32).ap()

    with tile.TileContext(nc) as tc:
        with (
            tc.tile_pool(name="wgt", bufs=1) as wpool,
            tc.tile_pool(name="qk", bufs=1) as qkpool,
            tc.tile_pool(name="vpr", bufs=1) as vpool,
            tc.tile_pool(name="pt", bufs=10) as ptpool,
            tc.tile_pool(name="yc", bufs=2) as ycpool,
            tc.tile_pool(name="ytp", bufs=2) as ytpool,
            tc.tile_pool(name="nrm", bufs=2) as nrmpool,
            tc.tile_pool(name="ob", bufs=3) as obpool,
            tc.tile_pool(name="scp", bufs=2, space="PSUM") as scpool,
            tc.tile_pool(name="avp", bufs=1, space="PSUM") as avpool,
            tc.tile_pool(name="trp", bufs=1, space="PSUM") as trpool,
            tc.tile_pool(name="prj", bufs=1, space="PSUM") as prpool,
        ):
            # ---- phase 0: load weights/constants ----
            # sync queue: only the QK-critical tensors, so scores start early;
            # everything else rides the scalar/gpsimd DMA queues in parallel
            wq = []   # fp8 wa-qk d-pair tiles [128, 2, 768]
            for t in range(NDP):
                wt = wpool.tile([P, 2 * 2 * DL], F8, tag=f"wq{t}")
                nc.sync.dma_start(wt[:], wq8[bass.ts(t, P), :])
                wq.append(wt)
            xq = []   # fp8 x d-pair tiles [128, 2, S]
            for t in range(NDP):
                xt = wpool.tile([P, 2 * S], F8, tag=f"xq{t}")
                nc.sync.dma_start(xt[:, 0:S], xq8[bass.ts(t, P), 0:S])
                nc.sync.dma_start(xt[:, S:], xq8[bass.ts(t, P), S:])
                xq.append(xt)
            wv = []
            for t in range(NDT):
                wt = wpool.tile([P, DL], BF16, tag=f"wv{t}")
                nc.gpsimd.dma_start(wt[:], wvb[bass.ts(t, P), :])
                wv.append(wt)
            wpp = []
            for p in range(NP):
                wt = wpool.tile([P, D], BF16, tag=f"wp{p}")
                nc.gpsimd.dma_start(wt[:], wpb[bass.ts(p, P), :])
                wpp.append(wt)
            ba_sb = wpool.tile([P, 8], F32, tag="ba")
            if qk_bias:
                nc.gpsimd.dma_start(ba_sb[:], ba8[:])
            bp_sb = wpool.tile([P, D], F32, tag="bp")
            if proj_bias:
                nc.gpsimd.dma_start(bp_sb[:], bpb[:])
            mt_sb = wpool.tile([P, P], BF16, tag="mtri")
            nc.gpsimd.dma_start(mt_sb[:], mtri[:])
            id_sb = wpool.tile([P, P], BF16, tag="idn")
            nc.gpsimd.dma_start(id_sb[:], idn[:])
            xk = []   # bf16 x^T tiles [128, S] (V stationary; only needed
            for t in range(NDT):  # from emit_v, so loaded after the consts)
                xt = wpool.tile([P, S], BF16, tag=f"xk{t}")
                nc.gpsimd.dma_start(xt[:, 0 : S // 2], xkb[bass.ts(t, P), 0 : S // 2])
                nc.gpsimd.dma_start(xt[:, S // 2 :], xkb[bass.ts(t, P), S // 2 :])
                xk.append(xt)
            # prewarm ScalarE's exp table
            warm_sb = wpool.tile([1, 2], F32, tag="warm")
            nc.vector.memset(warm_sb[:], 0.0)
            nc.scalar.activation(warm_sb[:, 1:2], warm_sb[:, 0:1],
                                 mybir.ActivationFunctionType.Exp)

            # ---- phase 1: Q,K -> fp8 [32,2,S]-layout tiles ----
            # qraw m-tiles 0..2 = Q pairs, 3..5 = K pairs, [128, S] fp8
            # (partitions: head A dims 0:64, head B dims 64:128)
            qt8 = [qkpool.tile([64, 2 * S], F8, tag=f"qt8{p}", name=f"qt8{p}") for p in range(NP)]
            kt8 = [qkpool.tile([64, 2 * S], F8, tag=f"kt8{p}", name=f"kt8{p}") for p in range(NP)]

            def emit_qk(m):
                qraw = qkpool.tile([P, S], F8, tag=f"qraw{m}")
                for n in range(S // QT):
                    ps = scpool.tile([P, 1024], F32, tag="sc")
                    for t in range(NDP):
                        nc.tensor.matmul(
                            ps[:, 0:QT],
                            lhsT=wq[t][:].rearrange("p (u m) -> p u m", u=2)[
                                :, :, bass.ts(m, P)],
                            rhs=xq[t][:].rearrange("p (u s) -> p u s", u=2)[
                                :, :, bass.ts(n, QT)],
                            start=(t == 0), stop=(t == NDP - 1), perf_mode=DR,
                        )
                    if qk_bias:
                        nc.vector.tensor_scalar_add(
                            qraw[:, bass.ts(n, QT)], ps[:, 0:QT], ba_sb[:, m : m + 1])
                    else:
                        nc.vector.tensor_copy(qraw[:, bass.ts(n, QT)], ps[:, 0:QT])
                # relayout [128, S] -> [32, 2, S] per head (A rows 0:32, B 32:64)
                dst = qt8[m] if m < NP else kt8[m - NP]
                # ACT's DMA queue is free during the head phase; use it for
                # the first pairs so the exp stream starts ASAP
                qmap = {0: [nc.sync, nc.scalar, nc.sync, nc.scalar],
                        3: [nc.scalar, nc.sync, nc.scalar, nc.sync],
                        1: [nc.scalar, nc.gpsimd, nc.sync, nc.gpsimd],
                        4: [nc.gpsimd, nc.sync, nc.gpsimd, nc.scalar],
                        2: [nc.sync, nc.gpsimd, nc.scalar, nc.sync],
                        5: [nc.gpsimd, nc.scalar, nc.sync, nc.gpsimd]}
                for h in range(2):
                    for u in range(2):
                        e = qmap[m][2 * h + u]
                        e.dma_start(
                            dst[32 * h : 32 * h + 32, u * S : (u + 1) * S],
                            qraw[64 * h + 32 * u : 64 * h + 32 * u + 32, :],
                        )

            # V s-tiles: [128, 780] fp8 = [hi 6x65 | lo 6x65], col 64-of-65:
            # hi=1 (softmax denominator via ones trick), lo=0
            v_t = [None] * NS

            def emit_v(s):
                ps = prpool.tile([P, 512], F32, tag="prj")
                for t in range(NDT):
                    nc.tensor.matmul(
                        ps[:, 0:DL],
                        lhsT=xk[t][:, bass.ts(s, P)],
                        rhs=wv[t][:],
                        start=(t == 0), stop=(t == NDT - 1),
                    )
                vt = vpool.tile([P, HL * 65], BF16, tag=f"v{s}")
                vt3 = vt[:].rearrange("p (h x) -> p h x", h=HL)
                ps3 = ps[:, 0:DL].rearrange("p (h x) -> p h x", h=HL)
                nc.vector.tensor_copy(vt3[:, :, 0:HD], ps3)
                nc.vector.memset(vt3[:, :, HD : HD + 1], 1.0)
                v_t[s] = vt

            ytp_t = [None] * NP

            def q_ap(p, h, q0, n):
                # rhs AP [32, 2, n] at q offset q0 for head h of pair p
                return qt8[p][32 * h : 32 * h + 32, :].rearrange(
                    "p (u s) -> p u s", u=2)[:, :, q0 : q0 + n]

            def k_ap(p, h, j):
                return kt8[p][32 * h : 32 * h + 32, :].rearrange(
                    "p (u s) -> p u s", u=2)[:, :, bass.ts(j, P)]

            def emit_attn_scores(i, p):
                ctx = {"fpts": [], "av": avpool.tile([P, 1024], F32, tag="av",
                                                     name=f"av{i}_{p}")}
                # off-diagonal full groups (pairs of j blocks)
                for g in range(2 * i):
                    j0, j1 = 2 * g, 2 * g + 1
                    pts = []
                    for h in range(2):
                        sc = scpool.tile([P, 1024], F32, tag="sc", name=f"scf{g}{h}")
                        for jj, off in ((j0, 0), (j1, QT)):
                            nc.tensor.matmul(
                                sc[:, off : off + QT],
                                lhsT=k_ap(p, h, jj),
                                rhs=q_ap(p, h, i * QT, QT),
                                start=True, stop=True, perf_mode=DR,
                            )
                        pt = ptpool.tile([P, 1024], BF16, tag="pt", name=f"ptf{g}{h}")
                        nc.scalar.activation(pt[:], sc[:],
                                             mybir.ActivationFunctionType.Exp,
                                             scale=ESC)
                        pts.append(pt)
                    ctx["fpts"].append((j0, j1, pts))

                # diagonal: 8 staircase chunks (A 0:512 | B 512:1024) + mask
                scd = scpool.tile([P, 1024], F32, tag="sc", name="scd")
                for h in range(2):
                    for mi in range(4):
                        nc.tensor.matmul(
                            scd[:, h * QT + mi * P : h * QT + mi * P + P],
                            lhsT=k_ap(p, h, 4 * i + mi),
                            rhs=q_ap(p, h, i * QT + mi * P, P),
                            start=(mi == 0), stop=False, perf_mode=DR,
                            skip_group_check=True,
                        )
                for h in range(2):
                    for mi in range(4):
                        # += mtri via PE: idn^T @ mtri = mtri (keeps the mask
                        # off DVE so exp never waits on the vector queue)
                        nc.tensor.matmul(
                            scd[:, h * QT + mi * P : h * QT + mi * P + P],
                            lhsT=id_sb[:], rhs=mt_sb[:],
                            start=False, stop=True, skip_group_check=True,
                        )
                ptD = ptpool.tile([P, 1024], BF16, tag="pt", name="ptD")
                nc.scalar.activation(ptD[:], scd[:],
                                     mybir.ActivationFunctionType.Exp, scale=ESC)
                ctx["ptD"] = ptD
                # diagonal remainders (mask-free): mi0@0 w384, mi2@384 w128,
                # mi1@512 w256  (bank-crossing-free packing)
                ptR = []
                for h in range(2):
                    scr = scpool.tile([P, 1024], F32, tag="sc", name=f"scr{h}")
                    for mi in range(3):
                        w = REM_W[mi]
                        off = REM_OFF[mi]
                        nc.tensor.matmul(
                            scr[:, off : off + w],
                            lhsT=k_ap(p, h, 4 * i + mi),
                            rhs=q_ap(p, h, i * QT + mi * P + P, w),
                            start=(mi < 2), stop=True, perf_mode=DR,
                            skip_group_check=True,
                        )
                    ptr = ptpool.tile([P, 1024], BF16, tag="pt", name=f"ptr{h}")
                    nc.scalar.activation(ptr[:, 0:768], scr[:, 0:768],
                                         mybir.ActivationFunctionType.Exp,
                                         scale=ESC)
                    ptR.append(ptr)
                ctx["ptR"] = ptR
                return ctx

            def emit_attn_av(i, p, ctx):
                av = ctx["av"]
                av_started = [False, False]

                def av_mm(h, cc, j, pt_ap, stop):
                    # start=True zero-fills the whole PSUM bank, so exactly
                    # the first matmul touching each head's bank carries it
                    hh = 2 * p + h
                    nc.tensor.matmul(
                        av[:, 512 * h + cc * 65 : 512 * h + cc * 65 + 65],
                        lhsT=pt_ap,
                        rhs=v_t[j][:, 65 * hh : 65 * hh + 65],
                        start=not av_started[h], stop=stop,
                        skip_group_check=True,
                    )
                    av_started[h] = True

                for j0, j1, pts in ctx["fpts"]:
                    for h in range(2):
                        for cc in range(4):
                            for jj, off in ((j0, 0), (j1, QT)):
                                av_mm(h, cc, jj,
                                      pts[h][:, off + cc * P : off + cc * P + P],
                                      stop=False)
                ptD, ptR = ctx["ptD"], ctx["ptR"]
                for h in range(2):
                    for cc in range(4):
                        for mi in range(cc):
                            av_mm(h, cc, 4 * i + mi,
                                  ptR[h][:, REM_OFF[mi] + (cc - mi - 1) * P :
                                         REM_OFF[mi] + (cc - mi) * P],
                                  stop=False)
                        av_mm(h, cc, 4 * i + cc,
                              ptD[:, h * QT + cc * P : h * QT + cc * P + P],
                              stop=True)

                # normalize (per-partition scalar) + build y chunks
                rc = nrmpool.tile([P, 8], F32, tag="rc")
                yc = ycpool.tile([P, QT], BF16, tag="yc")
                yc4 = yc[:].rearrange("p (c n) -> p c n", c=4)
                for h in range(2):
                    av4 = av[:, 512 * h : 512 * h + 260].rearrange(
                        "p (c n) -> p c n", c=4)
                    nc.vector.reciprocal(
                        rc[:, 4 * h : 4 * h + 4].unsqueeze(2), av4[:, :, 64:65])
                    nc.vector.tensor_mul(
                        yc4[:, :, 64 * h : 64 * h + 64],
                        av4[:, :, 0:64],
                        rc[:, 4 * h : 4 * h + 4].unsqueeze(2).broadcast_to((P, 4, 64)),
                    )
                ytp = ytpool.tile([P, QT], BF16, tag=f"ytp{p}")
                tp = trpool.tile([P, 1024], BF16, tag="tr")
                for cc in range(4):
                    nc.tensor.matmul(
                        tp[:, cc * P : cc * P + P], yc[:, cc * P : cc * P + P],
                        id_sb[:], is_transpose=True,
                        start=(cc == 0), stop=True, skip_group_check=True)
                nc.vector.tensor_copy(ytp[:], tp[:, 0:QT])
                ytp_t[p] = ytp

            def emit_rs(c):
                # ReduceScatter half c: rows [512c:512c+512) U [1024+512c:...)
                # as a 2-block AP; rank0 receives the first block (its own
                # rows), rank1 the second -> even core ends with rows
                # [0:1024), odd with [1024:2048); host concatenates.
                # c=0 fires after proj(2) (hidden); c=1 is the tail.
                nc.gpsimd.collective_compute(
                    "ReduceScatter",
                    mybir.AluOpType.add,
                    replica_groups=[[0, 1], [2, 3], [4, 5], [6, 7]],
                    ins=[ar_in[1024 * c : 1024 * c + 1024, :].opt()],
                    outs=[rs_buf[512 * c : 512 * c + 512, :].opt()],
                )
                if c == 0:
                    engs = [nc.sync, nc.gpsimd]
                    for blk in range(2):
                        r0 = 256 * blk
                        oc = obpool.tile([P, 2 * D], F32, tag="oc2",
                                         name=f"oc{c}{blk}")
                        e = engs[blk]
                        src = rs_buf[r0 : r0 + 256, :].rearrange(
                            "(c p) d -> p c d", c=2)
                        dst = outpart[r0 : r0 + 256, :].rearrange(
                            "(c p) d -> p c d", c=2)
                        e.dma_start(oc[:].rearrange("p (c d) -> p c d", c=2), src)
                        e.dma_start(dst, oc[:].rearrange("p (c d) -> p c d", c=2))
                else:
                    engs = [nc.sync, nc.scalar]
                    for blk in range(2):
                        r0 = 512 + 256 * blk
                        oc = obpool.tile([P, 2 * D], F32, tag="oc2",
                                         name=f"oc{c}{blk}")
                        e = engs[blk]
                        src = rs_buf[r0 : r0 + 256, :].rearrange(
                            "(c p) d -> p c d", c=2)
                        dst = outpart[r0 : r0 + 256, :].rearrange(
                            "(c p) d -> p c d", c=2)
                        e.dma_start(oc[:].rearrange("p (c d) -> p c d", c=2), src)
                        e.dma_start(dst, oc[:].rearrange("p (c d) -> p c d", c=2))

            AR_BLOCK = {0: 0, 2: 1, 1: 2, 3: 3}   # q-block -> ar_in block

            def emit_proj(i):
                for ss in range(4):
                    row = AR_BLOCK[i] * QT + ss * P
                    if proj_bias:
                        ps = prpool.tile([P, D], F32, tag="prjw")
                        for half in range(2):
                            for p in range(NP):
                                nc.tensor.matmul(
                                    ps[:, half * DL : half * DL + DL],
                                    lhsT=ytp_t[p][:, ss * P : ss * P + P],
                                    rhs=wpp[p][:, half * DL : half * DL + DL],
                                    start=(p == 0), stop=(p == NP - 1),
                                )
                        ob = obpool.tile([P, D], F32, tag="ob")
                        nc.vector.tensor_add(ob[:], ps[:], bp_sb[:])
                        nc.sync.dma_start(ar_in[row : row + P, :], ob[:])
                    else:
                        ob = obpool.tile([P, D], F32, tag="ob")
                        for half in range(2):
                            ps = prpool.tile([P, 512], F32, tag="prj")
                            for p in range(NP):
                                nc.tensor.matmul(
                                    ps[:, 0:DL],
                                    lhsT=ytp_t[p][:, ss * P : ss * P + P],
                                    rhs=wpp[p][:, half * DL : half * DL + DL],
                                    start=(p == 0), stop=(p == NP - 1),
                                )
                            nc.vector.tensor_copy(
                                ob[:, half * DL : half * DL + DL], ps[:, 0:DL])
                        nc.sync.dma_start(ar_in[row : row + P, :], ob[:])

            # ---- main schedule ----
            for m in (0, 3, 1, 4, 2, 5):
                emit_qk(m)
            ctxs = [emit_attn_scores(0, p) for p in range(NP)]
            for ss in range(4):
                emit_v(ss)
            emit_attn_av(0, 0, ctxs[0])
            emit_attn_av(0, 1, ctxs[1])
            for ss in range(4, 8):
                emit_v(ss)
            emit_attn_av(0, 2, ctxs[2])
            emit_proj(0)
            for i in range(1, NI):
                ctx0 = emit_attn_scores(i, 0)
                emit_attn_av(i, 0, ctx0)
                ctx1 = emit_attn_scores(i, 1)
                emit_attn_av(i, 1, ctx1)
                ctx2 = emit_attn_scores(i, 2)
                if i + 1 < NI:
                    for ss in range(4 * (i + 1), 4 * (i + 2)):
                        emit_v(ss)
                emit_attn_av(i, 2, ctx2)
                emit_proj(i)
                if i == 2:
                    emit_rs(0)
            emit_rs(1)

    _legalize_waits(nc)
    return nc


_NC_CACHE = {}


def _get_nc(qk_bias=False, proj_bias=False):
    key = (qk_bias, proj_bias)
    if key not in _NC_CACHE:
        _NC_CACHE[key] = _build(qk_bias, proj_bias)
    return _NC_CACHE[key]


def _prep_inputs(x, W_attn, b_attn, W_proj, b_proj):
    bf = ml_dtypes.bfloat16
    f8 = ml_dtypes.float8_e4m3
    x = np.asarray(x, np.float32)
    W_attn = np.asarray(W_attn, np.float32)
    b_attn = np.asarray(b_attn, np.float32)
    W_proj = np.asarray(W_proj, np.float32)
    b_proj = np.asarray(b_proj, np.float32)

    k_idx = np.arange(P)[:, None]
    q_idx = np.arange(P)[None, :]
    mtri = np.where(q_idx >= k_idx, 0.0, MASKC).astype(bf)
    idn = np.eye(P).astype(bf)

    Wq = W_attn[:, 0:D]
    Wk = W_attn[:, D : 2 * D]
    Wv = W_attn[:, 2 * D :]

    in_maps = []
    meta = {}
    for c in range(N_CORES):
        b, g = divmod(c, 2)
        cols = slice(DL * g, DL * g + DL)
        xb = x[b]                                   # [S, D]
        # fp8 x in d-pair layout [3*128, 2*S]
        xq8 = np.empty((3 * P, 2 * S), f8)
        for t in range(NDP):
            for u in range(2):
                xq8[t * P : (t + 1) * P, u * S : (u + 1) * S] = (
                    xb[:, 256 * t + P * u : 256 * t + P * u + P].T.astype(f8))
        xkb = np.ascontiguousarray(xb.T).astype(bf)
        # fp8 W_attn q,k (scaled) in d-pair layout [3*128, 2*768]
        wa_qk = np.concatenate([Wq[:, cols], Wk[:, cols]], axis=1) * SA  # [D, 768]
        wq8 = np.empty((3 * P, 2 * 2 * DL), f8)
        for t in range(NDP):
            for u in range(2):
                wq8[t * P : (t + 1) * P, u * 2 * DL : (u + 1) * 2 * DL] = (
                    wa_qk[256 * t + P * u : 256 * t + P * u + P, :].astype(f8))
        wvb = np.ascontiguousarray(Wv[:, cols]).astype(bf)
        wpb = np.ascontiguousarray(W_proj[cols, :]).astype(bf)
        # qk bias (scaled): ba8[p, m] = SA * b[col m*128+p]
        ba_qk = np.concatenate([b_attn[0:D][cols], b_attn[D : 2 * D][cols]]) * SA
        ba8 = np.zeros((P, 8), np.float32)
        ba8[:, :6] = ba_qk.reshape(6, P).T
        # b_proj (+ v-bias folded) added once per row: only group 0 carries it
        bv = b_attn[2 * D :][cols]
        bp_eff = bv @ W_proj[cols, :] + (b_proj if g == 0 else 0.0)
        bpb = np.ascontiguousarray(
            np.broadcast_to(bp_eff.astype(np.float32), (P, D)))
        in_maps.append({
            "xq8": xq8, "xkb": xkb, "wq8": wq8, "wvb": wvb, "wpb": wpb,
            "ba8": ba8, "bpb": bpb, "mtri": mtri, "idn": idn,
        })
        meta.setdefault("qk_bias", bool(np.any(ba_qk != 0.0)))
        meta["proj_bias"] = meta.get("proj_bias", False) or bool(
            np.any(bp_eff != 0.0))
    return in_maps, meta


def kernel(x, W_attn, b_attn, W_proj, b_proj):
    in_maps, meta = _prep_inputs(x, W_attn, b_attn, W_proj, b_proj)
    nc = _get_nc(meta["qk_bias"], meta["proj_bias"])
    res = run_bass_kernel_spmd(nc, in_maps, list(range(N_CORES)))
    out = np.stack([
        np.concatenate(
            [res.results[2 * b]["outpart"], res.results[2 * b + 1]["outpart"]])
        for b in range(B)
    ])
    return out.astype(np.float32)
